# revision 35
# baseline (speedup 1.0000x reference)
"""Trainium2 Bass kernel for EquivariantAttention (sparse_attention).

Full (unsharded) inputs in, full output out. Internally shards over the 8
NeuronCores as (batch, T-half): core c handles batch b = c // 2, query rows
t0 = (c % 2) * 256 .. t0+256.  Every core runs the identical SPMD program on
its own input slices; there is no cross-core communication (LN and out_proj
are row-local in (b, t)).

Device-side per core:
  scores_T[s,t] = bias_T + (k_T.T @ q_T)          (bias preloaded into PSUM
                                                   via identity matmul, QK
                                                   accumulates on top, fp32r)
  m = (scores_T + 20) * law_T                     (one DVE pass, fp32)
  e = exp(m - 20)  -> bf16                        (ACT, free affine bias)
  g = e * law_T                                   (DVE, bf16)
  denom[t] = sum_s e                              (ones-column matmul)
  numer[j,t] = sum_s v_ext[s,j] * g[s,t]          (bf16 matmul; V PBC rows
                                                   gathered on device by
                                                   indirect DMA)
  attn = numer * recip(denom)                     (DVE)
  inorm = rsqrt(SSQ/512 + 1e-3)                   (SSQ via ones-matmul of
                                                   attn^2, ACT rsqrt +
                                                   1 Newton step)
  out = (attn @ (out_proj_w * ln_w).T) * inorm    (fp32r matmuls, inorm
                                                   folded into PSUM->SBUF
                                                   copy-out)
"""

import numpy as np
import ml_dtypes

import concourse.bass as bass
import concourse.bacc as bacc
import concourse.tile as tile
from concourse import mybir
from concourse.bass_utils import run_bass_kernel_spmd

# Problem constants (hardcoded per contract)
B, T, P, HID = 4, 512, 3, 512
H, D = 16, 32
EXP = 256
S = T + EXP            # 768
SCALING = (D / 3.0) ** 0.5 / D
SMOOTH = 20.0
EPS = 1e-3

NCORES = 8
TQ = T // 2            # 256 query rows per core
DH = P * D             # 96 head dim
NST = S // 128         # 6 s-tiles of 128
HG = 4                 # head groups of 4 heads

F32 = mybir.dt.float32
F16 = mybir.dt.float16
F32R = mybir.dt.float32r
BF16 = mybir.dt.bfloat16
I32 = mybir.dt.int32
AF = mybir.ActivationFunctionType
ALU = mybir.AluOpType

_CACHED_NC = None


def r(ap):
    """bitcast f32 AP -> f32r for full-rate PE."""
    return ap.bitcast(F32R)


def build_nc():
    nc = bacc.Bacc("TRN2", target_bir_lowering=False, debug=False)

    # ---- DRAM I/O (per-core shapes) ----
    d_bias = nc.dram_tensor("biasT", [S, H, TQ], F16, kind="ExternalInput").ap()
    d_law = nc.dram_tensor("lawT", [128, NST * TQ], F32, kind="ExternalInput").ap()
    d_lawb = nc.dram_tensor("lawTb", [128, NST * TQ], BF16, kind="ExternalInput").ap()
    d_qT = nc.dram_tensor("qT", [DH, H * TQ], BF16, kind="ExternalInput").ap()
    d_kTe = nc.dram_tensor("kTe", [DH, H * S], BF16, kind="ExternalInput").ap()
    d_vb = nc.dram_tensor("vb", [T, P * HID], BF16, kind="ExternalInput").ap()
    d_vidx = nc.dram_tensor("vidx", [2, 128, 1], I32, kind="ExternalInput").ap()
    d_wT = nc.dram_tensor("wT", [HID, HID], F16, kind="ExternalInput").ap()
    d_id = nc.dram_tensor("ident", [128, 128], F16, kind="ExternalInput").ap()
    d_out = nc.dram_tensor("out", [TQ, P, HID], F32, kind="ExternalOutput").ap()
    d_rec = nc.dram_tensor("rec_scratch", [16, TQ], F32).ap()
    d_ino = nc.dram_tensor("inorm_scratch", [TQ], F32).ap()

    with tile.TileContext(nc) as tc:
        build_kernel(tc, d_bias, d_law, d_lawb, d_qT, d_kTe, d_vb, d_vidx,
                     d_wT, d_id, d_out, d_rec, d_ino)
    nc.compile()
    return nc


def build_kernel(tc, d_bias, d_law, d_lawb, d_qT, d_kTe, d_vb, d_vidx,
                 d_wT, d_id, d_out, d_rec, d_ino):
    nc = tc.nc
    from contextlib import ExitStack
    ctx = ExitStack()
    with ctx:
        const = ctx.enter_context(tc.tile_pool(name="const", bufs=1))
        big = ctx.enter_context(tc.tile_pool(name="big", bufs=1))
        biasp = ctx.enter_context(tc.tile_pool(name="biasp", bufs=4))
        work = ctx.enter_context(tc.tile_pool(name="work", bufs=2))
        attnp = ctx.enter_context(tc.tile_pool(name="attnp", bufs=1))
        psum = ctx.enter_context(tc.tile_pool(name="psum", bufs=2, space="PSUM"))
        psum1 = ctx.enter_context(tc.tile_pool(name="psum1", bufs=1, space="PSUM"))

        # ---- constants ----
        ident = const.tile([128, 128], F16, tag="ident")
        nc.sync.dma_start(out=ident[:], in_=d_id)
        ones_b = const.tile([128, 1], BF16, tag="ones_b")
        nc.vector.memset(ones_b[:], 1.0)
        ones_f = const.tile([128, 1], F32, tag="ones_f")
        nc.vector.memset(ones_f[:], 1.0)
        neg20 = const.tile([128, 1], F32, tag="neg20")
        nc.vector.memset(neg20[:], -SMOOTH)
        ones_r = const.tile([1, 1], F32R, tag="ones_r")
        nc.vector.tensor_copy(ones_r[:], ones_f[0:1, 0:1])

        # ---- resident loads ----
        law = const.tile([128, NST * TQ], F32, tag="law")      # (s%128,(st,t))
        lawb = const.tile([128, NST * TQ], BF16, tag="lawb")
        qT = const.tile([DH, H * TQ], BF16, tag="qT")
        kTe = big.tile([DH, H * S], BF16, tag="kTe")
        # need-ordered: first head-group's k/q first, law early (stt dep)
        with tc.high_priority():
            nc.sync.dma_start(out=law[:], in_=d_law)
            nc.sync.dma_start(out=kTe[:, :4 * S], in_=d_kTe[:, :4 * S])
            nc.sync.dma_start(out=qT[:, :4 * TQ], in_=d_qT[:, :4 * TQ])
            nc.sync.dma_start(out=lawb[:], in_=d_lawb)
        nc.sync.dma_start(out=qT[:, 4 * TQ:8 * TQ],
                          in_=d_qT[:, 4 * TQ:8 * TQ])
        for c in range(1, 4):
            nc.sync.dma_start(
                out=kTe[:, c * 4 * S:(c + 1) * 4 * S],
                in_=d_kTe[:, c * 4 * S:(c + 1) * 4 * S])
            if c == 1:
                nc.sync.dma_start(out=qT[:, 8 * TQ:], in_=d_qT[:, 8 * TQ:])
        wT = const.tile([128, 4 * HID], F16, tag="wT")          # (c%128,(ci,o))

        # V tiles: 4 direct + 2 gathered (PBC expansion), bf16
        v_sb = []
        for st in range(4):
            vt = const.tile([128, P * HID + 128], BF16, tag=f"v{st}",
                            name=f"v{st}")
            nc.vector.memset(vt[:, P * HID:], 0.0)
            from contextlib import nullcontext
            with tc.high_priority(offset=10000) if st < 3 else nullcontext():
                nc.sync.dma_start(out=vt[:, :P * HID],
                                  in_=d_vb[st * 128:(st + 1) * 128, :])
            v_sb.append(vt)
        idx_sb = const.tile([128, 2], I32, tag="idx")
        nc.gpsimd.dma_start(
            out=idx_sb[:].rearrange("p (two one) -> p two one", one=1),
            in_=d_vidx.rearrange("two p one -> p two one"))
        for gi in range(2):
            vt = const.tile([128, P * HID + 128], BF16, tag=f"v{4 + gi}",
                            name=f"vg{gi}")
            nc.vector.memset(vt[:, P * HID:], 0.0)
            nc.gpsimd.indirect_dma_start(
                out=vt[:, :P * HID], out_offset=None,
                in_=d_vb[:, :],
                in_offset=bass.IndirectOffsetOnAxis(
                    ap=idx_sb[:, gi:gi + 1], axis=0))
            v_sb.append(vt)

        # recip workspace
        rec2 = const.tile([1, 2 * TQ], F32, tag="rec2")
        rscr = const.tile([1, 2 * TQ], F32, tag="rscr")

        # attention outputs (divided), one tile per head-pair: [96, 2*TQ]
        apair = []
        for h2 in range(H // 2):
            apair.append(attnp.tile([DH, 2 * TQ], F16, tag=f"apair{h2}",
                                    name=f"apair{h2}"))

        # ================= attention main loop =================
        for hg in range(HG):
            # per-hg psum accumulators (live across st loop)
            numer = [psum1.tile([128, 2 * TQ], F32, space="PSUM",
                                tag=f"numer_{i}",
                                name=f"numer{hg}_{i}") for i in range(2)]
            den_ps = [psum1.tile([1, 2 * TQ], F32, space="PSUM",
                                 tag=f"den_{i}", name=f"den{hg}_{i}")
                      for i in range(2)]
            for st in range(NST):
                scores = psum.tile([128, 4 * TQ], F32, space="PSUM",
                                   tag="scores")
                # bias preload into PSUM (identity matmul, fp32r full rate)
                bt = biasp.tile([128, 4 * TQ], F16, tag="bias")
                from contextlib import nullcontext
                with (tc.high_priority() if (hg == 0 and st < 2)
                      else nullcontext()):
                    nc.sync.dma_start(
                        out=bt[:].rearrange("p (i t) -> p i t", i=4),
                        in_=d_bias[st * 128:(st + 1) * 128,
                                   hg * 4:hg * 4 + 4, :])
                for half in range(2):
                    nc.tensor.matmul(
                        out=scores[:, half * 512:(half + 1) * 512],
                        lhsT=ident[:],
                        rhs=bt[:, half * 512:(half + 1) * 512],
                        start=True, stop=False)
                # QK accumulate on top (i=1,3 close their banks)
                for i in range(4):
                    h = hg * 4 + i
                    nc.tensor.matmul(
                        out=scores[:, i * TQ:(i + 1) * TQ],
                        lhsT=kTe[:, h * S + st * 128:h * S + (st + 1) * 128],
                        rhs=qT[:, h * TQ:(h + 1) * TQ],
                        start=False, stop=(i % 2 == 1))
                # m = (scores + 20) * law    [one fat DVE pass]
                m = work.tile([128, 4 * TQ], F32, tag="m")
                law_st = law[:, st * TQ:(st + 1) * TQ]
                nc.vector.scalar_tensor_tensor(
                    out=m[:].rearrange("p (i t) -> p i t", i=4),
                    in0=scores[:].rearrange("p (i t) -> p i t", i=4),
                    scalar=SMOOTH, in1=law_st.unsqueeze(1).to_broadcast([128, 4, TQ]),
                    op0=ALU.add, op1=ALU.mult)
                # e = exp(m - 20) -> bf16
                e = work.tile([128, 4 * TQ], BF16, tag="e")
                nc.scalar.activation(e[:], m[:], AF.Exp, bias=neg20[:], scale=1.0)
                # g = e * law (bf16)
                g = work.tile([128, 4 * TQ], BF16, tag="g")
                lawb_st = lawb[:, st * TQ:(st + 1) * TQ]
                geng = nc.gpsimd if st % 2 == 0 else nc.vector
                geng.tensor_tensor(
                    out=g[:].rearrange("p (i t) -> p i t", i=4),
                    in0=e[:].rearrange("p (i t) -> p i t", i=4),
                    in1=lawb_st.unsqueeze(1).to_broadcast([128, 4, TQ]),
                    op=ALU.mult)
                # denominators: ones.T @ e -> [1, 512] per head-pair
                for j in range(2):
                    nc.tensor.matmul(
                        out=den_ps[j][0:1, :],
                        lhsT=ones_b[:],
                        rhs=e[:, j * 512:(j + 1) * 512],
                        start=(st == 0), stop=(st == NST - 1))
                # numerators: v_ext.T @ g -> [96, TQ] per head
                for i in range(4):
                    h = hg * 4 + i
                    nc.tensor.matmul(
                        out=numer[i // 2][:, (i % 2) * TQ:(i % 2 + 1) * TQ],
                        lhsT=v_sb[st][:, h * DH:h * DH + 128],
                        rhs=g[:, i * TQ:(i + 1) * TQ],
                        start=(st == 0 and i % 2 == 0),
                        stop=(st == NST - 1 and i % 2 == 1))
            # drain: recip -> DRAM -> broadcast, divide fused into the
            # numer psum->sbuf copy (divided fp16 apairs)
            h0 = hg * 4
            for i in range(2):
                nc.vector.reciprocal_approx_fast(
                    out=rec2[:, :], in_=den_ps[i][0:1, :])
                nc.sync.dma_start(
                    out=d_rec[h0 + 2 * i:h0 + 2 * i + 2, :],
                    in_=rec2[:, :].rearrange("one (j t) -> one j t", j=2))
                rrep = work.tile([DH, 2 * TQ], F32, tag="rrep")
                for j in range(2):
                    nc.scalar.dma_start(
                        out=rrep[:, j * TQ:(j + 1) * TQ].unsqueeze(1),
                        in_=d_rec[h0 + 2 * i + j:h0 + 2 * i + j + 1, :]
                            .unsqueeze(1).to_broadcast((DH, 1, TQ)))
                nc.vector.tensor_tensor(
                    out=apair[hg * 2 + i][:], in0=numer[i][:96, :],
                    in1=rrep[:], op=ALU.mult)

        # ============ remap heads -> channel-major tiles ============
        # attn_ct[ci] : [128 (c%128), P*TQ], c = h*32+dd, free = (p, t)
        attn_ct = []
        for ci in range(4):
            act = attnp.tile([128, P * TQ], F16, tag=f"act{ci}", name=f"act{ci}")
            attn_ct.append(act)
        for h in range(H):
            ci, r0 = h // 4, (h % 4) * 32
            for p in range(P):
                eng = (nc.sync, nc.scalar, nc.gpsimd)[(h * P + p) % 3]
                eng.dma_start(
                    out=attn_ct[ci][r0:r0 + 32, p * TQ:(p + 1) * TQ],
                    in_=apair[h // 2][p * 32:p * 32 + 32,
                                      (h % 2) * TQ:(h % 2 + 1) * TQ])

        # ================= equivariant LN =================
        sqp = work.tile([128, P * TQ], F32R, tag="sq")
        ssq_a = psum1.tile([1, 512], F32, space="PSUM", tag="den_0")
        ssq_b = psum1.tile([1, TQ], F32, space="PSUM", tag="den_1")
        for ci in range(4):
            aci = attn_ct[ci][:]
            nc.vector.tensor_tensor(out=sqp[:], in0=aci, in1=aci, op=ALU.mult)
            nc.tensor.matmul(out=ssq_a[0:1, :], lhsT=ones_f[:].bitcast(F32R),
                             rhs=sqp[:, 0:512],
                             start=(ci == 0), stop=(ci == 3))
            nc.tensor.matmul(out=ssq_b[0:1, :], lhsT=ones_f[:].bitcast(F32R),
                             rhs=sqp[:, 512:768],
                             start=(ci == 0), stop=(ci == 3))
        # fold p-blocks: y[t] = ssq(p0)+ssq(p1)+ssq(p2)
        yrow = const.tile([1, TQ], F32, tag="yrow")
        nc.vector.tensor_copy(yrow[:], ssq_a[0:1, 0:TQ])
        nc.vector.tensor_tensor(out=yrow[:], in0=yrow[:],
                                in1=ssq_a[0:1, TQ:2 * TQ], op=ALU.add)
        nc.vector.tensor_tensor(out=yrow[:], in0=yrow[:], in1=ssq_b[0:1, :],
                                op=ALU.add)
        # inorm = rsqrt(y/512 + eps), then one Newton step
        # r' = r*(1.5 - 0.5*a*r^2) with a = y/512+eps
        arow = const.tile([1, TQ], F32, tag="arow")
        nc.vector.tensor_scalar(
            out=arow[:], in0=yrow[:], scalar1=1.0 / HID, scalar2=EPS,
            op0=ALU.mult, op1=ALU.add)
        rcpa = const.tile([1, TQ], F32, tag="rcpa")
        nc.vector.reciprocal_approx_fast(out=rcpa[:], in_=arow[:])
        r0t = const.tile([1, TQ], F32, tag="r0t")
        nc.scalar.activation(r0t[:], rcpa[:], AF.Sqrt, bias=0.0, scale=1.0)
        tmp = const.tile([1, TQ], F32, tag="tmpn")
        nc.vector.tensor_tensor(out=tmp[:], in0=r0t[:], in1=r0t[:], op=ALU.mult)
        nc.vector.tensor_tensor(out=tmp[:], in0=tmp[:], in1=arow[:], op=ALU.mult)
        nc.vector.tensor_scalar(
            out=tmp[:], in0=tmp[:], scalar1=-0.5, scalar2=1.5,
            op0=ALU.mult, op1=ALU.add)
        inorm = const.tile([1, TQ], F32R, tag="inorm")
        nc.vector.tensor_tensor(out=inorm[:], in0=r0t[:], in1=tmp[:],
                                op=ALU.mult)
        # inorm as columns [128,1] per t-half (bounce through DRAM)
        icol = const.tile([128, 2], F32, tag="icol")
        nc.sync.dma_start(out=d_ino.rearrange("(one t) -> one t", one=1),
                          in_=inorm[:, :].bitcast(F32))
        for th in range(2):
            nc.sync.dma_start(
                out=icol[:, th:th + 1],
                in_=d_ino[th * 128:(th + 1) * 128]
                    .rearrange("(p one) -> p one", one=1))

        # ================= out_proj =================
        nc.sync.dma_start(
            out=wT[:].rearrange("p (ci o) -> p ci o", ci=4),
            in_=d_wT.rearrange("(ci p) o -> p ci o", p=128))
        for k in range(6):          # tp-tiles: p = k//2, t-half = k%2
            op = psum.tile([128, HID], F32, space="PSUM", tag="scores")
            for ci in range(4):
                nc.tensor.matmul(
                    out=op[:, :],
                    lhsT=attn_ct[ci][:, k * 128:(k + 1) * 128],
                    rhs=wT[:, ci * HID:(ci + 1) * HID],
                    start=(ci == 0), stop=(ci == 3))
            ot = work.tile([128, HID], F32, tag="osb")
            nc.vector.tensor_scalar(
                out=ot[:], in0=op[:, :], scalar1=icol[:, k % 2:k % 2 + 1],
                scalar2=None, op0=ALU.mult)
            nc.sync.dma_start(
                out=d_out[(k % 2) * 128:(k % 2) * 128 + 128, k // 2, :],
                in_=ot[:])


def _host_prep(q, k, v, attn_bias, local_attention_weight, out_proj_w,
               ln_weight, outcell_index):
    """Pure layout marshalling on host -> per-core input dicts."""
    q = np.asarray(q, np.float32)
    k = np.asarray(k, np.float32)
    v = np.asarray(v, np.float32)
    attn_bias = np.asarray(attn_bias, np.float32)
    law = np.asarray(local_attention_weight, np.float32)
    out_proj_w = np.asarray(out_proj_w, np.float32)
    ln_weight = np.asarray(ln_weight, np.float32)
    idx = np.asarray(outcell_index).astype(np.int64)

    # (B,T,P,HID) -> (B, 96, H, T) with row j = p*32+dd
    def to_dT(x):
        return np.ascontiguousarray(
            x.reshape(B, T, P, H, D).transpose(0, 2, 4, 3, 1)
        ).reshape(B, P * D, H, T)

    qT = to_dT(q) * np.float32(SCALING)
    kT = to_dT(k)
    # K PBC expansion along token axis (gather columns)
    kTe = np.concatenate(
        [kT, np.take_along_axis(
            kT, idx[:, None, None, :].astype(np.int64), axis=3)], axis=3)
    biasT = np.ascontiguousarray(
        attn_bias.transpose(0, 3, 1, 2)).astype(np.float16)       # (B,S,H,T)
    lawT = np.ascontiguousarray(law.transpose(0, 2, 1))            # (B,S,T)
    lawTb = lawT.astype(ml_dtypes.bfloat16)
    # head-major V columns: (B, T, (h, p, dd)) so each head is contiguous
    vb = np.ascontiguousarray(
        v.reshape(B, T, P, H, D).transpose(0, 1, 3, 2, 4)
    ).reshape(B, T, P * HID).astype(ml_dtypes.bfloat16)
    wT = np.ascontiguousarray(out_proj_w.T) * ln_weight[:, None]   # (c,o)
    wT = np.ascontiguousarray(wT, np.float32).astype(np.float16)
    vidx = idx.astype(np.int32).reshape(B, 2, 128, 1)

    in_maps = []
    for c in range(NCORES):
        b, th = c // 2, c % 2
        t0 = th * TQ
        lawc = np.ascontiguousarray(lawT[b, :, t0:t0 + TQ])
        lawc = np.ascontiguousarray(
            lawc.reshape(NST, 128, TQ).transpose(1, 0, 2)).reshape(128, NST * TQ)
        lawcb = np.ascontiguousarray(lawTb[b, :, t0:t0 + TQ])
        lawcb = np.ascontiguousarray(
            lawcb.reshape(NST, 128, TQ).transpose(1, 0, 2)).reshape(128, NST * TQ)
        in_maps.append(dict(
            biasT=np.ascontiguousarray(biasT[b, :, :, t0:t0 + TQ]),
            lawT=lawc,
            lawTb=lawcb,
            qT=np.ascontiguousarray(
                qT[b, :, :, t0:t0 + TQ]).reshape(DH, H * TQ)
                .astype(ml_dtypes.bfloat16),
            kTe=np.ascontiguousarray(kTe[b]).reshape(DH, H * S)
                .astype(ml_dtypes.bfloat16),
            vb=np.ascontiguousarray(vb[b]),
            vidx=np.ascontiguousarray(vidx[b]),
            wT=wT,
            ident=np.eye(128, dtype=np.float16),
        ))
    return in_maps


def kernel(**inputs):
    global _CACHED_NC
    if _CACHED_NC is None:
        _CACHED_NC = build_nc()
    nc = _CACHED_NC
    in_maps = _host_prep(
        inputs["q"], inputs["k"], inputs["v"], inputs["attn_bias"],
        inputs["local_attention_weight"], inputs["out_proj_w"],
        inputs["ln_weight"], inputs["outcell_index"])
    res = run_bass_kernel_spmd(nc, in_maps, core_ids=list(range(NCORES)))
    out = np.empty((B, T, P, HID), np.float32)
    for c in range(NCORES):
        b, th = c // 2, c % 2
        out[b, th * TQ:(th + 1) * TQ] = res.results[c]["out"]
    return out


# revision 36
# speedup vs baseline: 1.1163x; 1.1163x over previous
"""Trainium2 Bass kernel for EquivariantAttention (sparse_attention).

Full (unsharded) inputs in, full output out. Internally shards over the 8
NeuronCores as (batch, T-half): core c handles batch b = c // 2, query rows
t0 = (c % 2) * 256 .. t0+256.  Every core runs the identical SPMD program on
its own input slices; there is no cross-core communication (LN and out_proj
are row-local in (b, t)).

Device-side per core:
  scores_T[s,t] = bias_T + (k_T.T @ q_T)          (bias preloaded into PSUM
                                                   via identity matmul, QK
                                                   accumulates on top, fp32r)
  m = (scores_T + 20) * law_T                     (one DVE pass, fp32)
  e = exp(m - 20)  -> bf16                        (ACT, free affine bias)
  g = e * law_T                                   (DVE, bf16)
  denom[t] = sum_s e                              (ones-column matmul)
  numer[j,t] = sum_s v_ext[s,j] * g[s,t]          (bf16 matmul; V PBC rows
                                                   gathered on device by
                                                   indirect DMA)
  attn = numer * recip(denom)                     (DVE)
  inorm = rsqrt(SSQ/512 + 1e-3)                   (SSQ via ones-matmul of
                                                   attn^2, ACT rsqrt +
                                                   1 Newton step)
  out = (attn @ (out_proj_w * ln_w).T) * inorm    (fp32r matmuls, inorm
                                                   folded into PSUM->SBUF
                                                   copy-out)
"""

import numpy as np
import ml_dtypes

import concourse.bass as bass
import concourse.bacc as bacc
import concourse.tile as tile
from concourse import mybir
from concourse.bass_utils import run_bass_kernel_spmd

# Problem constants (hardcoded per contract)
B, T, P, HID = 4, 512, 3, 512
H, D = 16, 32
EXP = 256
S = T + EXP            # 768
SCALING = (D / 3.0) ** 0.5 / D
SMOOTH = 20.0
EPS = 1e-3

NCORES = 8
TQ = T // 2            # 256 query rows per core
DH = P * D             # 96 head dim
NST = S // 128         # 6 s-tiles of 128
HG = 4                 # head groups of 4 heads

F32 = mybir.dt.float32
F16 = mybir.dt.float16
F32R = mybir.dt.float32r
BF16 = mybir.dt.bfloat16
I32 = mybir.dt.int32
AF = mybir.ActivationFunctionType
ALU = mybir.AluOpType

_CACHED_NC = None


def r(ap):
    """bitcast f32 AP -> f32r for full-rate PE."""
    return ap.bitcast(F32R)


def build_nc():
    nc = bacc.Bacc("TRN2", target_bir_lowering=False, debug=False)

    # ---- DRAM I/O (per-core shapes) ----
    d_bias = nc.dram_tensor("biasT", [S, H, TQ], F16, kind="ExternalInput").ap()
    d_law = nc.dram_tensor("lawT", [128, NST * TQ], F32, kind="ExternalInput").ap()
    d_lawb = nc.dram_tensor("lawTb", [128, NST * TQ], BF16, kind="ExternalInput").ap()
    d_qT = nc.dram_tensor("qT", [DH, H * TQ], BF16, kind="ExternalInput").ap()
    d_kTe = nc.dram_tensor("kTe", [DH, H * S], BF16, kind="ExternalInput").ap()
    d_vb = nc.dram_tensor("vb", [T, P * HID], BF16, kind="ExternalInput").ap()
    d_vidx = nc.dram_tensor("vidx", [2, 128, 1], I32, kind="ExternalInput").ap()
    d_wT = nc.dram_tensor("wT", [HID, HID], F16, kind="ExternalInput").ap()
    d_id = nc.dram_tensor("ident", [128, 128], F16, kind="ExternalInput").ap()
    d_out = nc.dram_tensor("out", [TQ, P, HID], F32, kind="ExternalOutput").ap()
    d_rec = nc.dram_tensor("rec_scratch", [16, TQ], F32).ap()
    d_ino = nc.dram_tensor("inorm_scratch", [TQ], F32).ap()

    with tile.TileContext(nc) as tc:
        build_kernel(tc, d_bias, d_law, d_lawb, d_qT, d_kTe, d_vb, d_vidx,
                     d_wT, d_id, d_out, d_rec, d_ino)
    nc.compile()
    return nc


def build_kernel(tc, d_bias, d_law, d_lawb, d_qT, d_kTe, d_vb, d_vidx,
                 d_wT, d_id, d_out, d_rec, d_ino):
    nc = tc.nc
    from contextlib import ExitStack
    ctx = ExitStack()
    with ctx:
        const = ctx.enter_context(tc.tile_pool(name="const", bufs=1))
        big = ctx.enter_context(tc.tile_pool(name="big", bufs=1))
        biasp = ctx.enter_context(tc.tile_pool(name="biasp", bufs=4))
        work = ctx.enter_context(tc.tile_pool(name="work", bufs=2))
        attnp = ctx.enter_context(tc.tile_pool(name="attnp", bufs=1))
        psum = ctx.enter_context(tc.tile_pool(name="psum", bufs=2, space="PSUM"))
        psum1 = ctx.enter_context(tc.tile_pool(name="psum1", bufs=1, space="PSUM"))

        # ---- constants ----
        ident = const.tile([128, 128], F16, tag="ident")
        nc.sync.dma_start(out=ident[:], in_=d_id)
        ones_b = const.tile([128, 1], BF16, tag="ones_b")
        nc.vector.memset(ones_b[:], 1.0)
        ones_f = const.tile([128, 1], F32, tag="ones_f")
        nc.vector.memset(ones_f[:], 1.0)
        neg20 = const.tile([128, 1], F32, tag="neg20")
        nc.vector.memset(neg20[:], -SMOOTH)
        ones_r = const.tile([1, 1], F32R, tag="ones_r")
        nc.vector.tensor_copy(ones_r[:], ones_f[0:1, 0:1])

        # ---- resident loads ----
        law = const.tile([128, NST * TQ], F32, tag="law")      # (s%128,(st,t))
        lawb = const.tile([128, NST * TQ], BF16, tag="lawb")
        qT = const.tile([DH, H * TQ], BF16, tag="qT")
        kTe = big.tile([DH, H * S], BF16, tag="kTe")
        # need-ordered: first head-group's k/q first, law early (stt dep)
        with tc.high_priority():
            nc.sync.dma_start(out=law[:], in_=d_law)
            nc.sync.dma_start(out=kTe[:, :4 * S], in_=d_kTe[:, :4 * S])
            nc.sync.dma_start(out=qT[:, :4 * TQ], in_=d_qT[:, :4 * TQ])
            nc.sync.dma_start(out=lawb[:], in_=d_lawb)
        nc.sync.dma_start(out=qT[:, 4 * TQ:8 * TQ],
                          in_=d_qT[:, 4 * TQ:8 * TQ])
        for c in range(1, 4):
            nc.sync.dma_start(
                out=kTe[:, c * 4 * S:(c + 1) * 4 * S],
                in_=d_kTe[:, c * 4 * S:(c + 1) * 4 * S])
            if c == 1:
                nc.sync.dma_start(out=qT[:, 8 * TQ:], in_=d_qT[:, 8 * TQ:])
        wT = const.tile([128, 4 * HID], F16, tag="wT")          # (c%128,(ci,o))

        # V tiles: 4 direct + 2 gathered (PBC expansion), bf16
        v_sb = []
        for st in range(4):
            vt = const.tile([128, P * HID + 128], BF16, tag=f"v{st}",
                            name=f"v{st}")
            nc.vector.memset(vt[:, P * HID:], 0.0)
            from contextlib import nullcontext
            with tc.high_priority(offset=10000) if st < 3 else nullcontext():
                nc.sync.dma_start(out=vt[:, :P * HID],
                                  in_=d_vb[st * 128:(st + 1) * 128, :])
            v_sb.append(vt)
        idx_sb = const.tile([128, 2], I32, tag="idx")
        nc.gpsimd.dma_start(
            out=idx_sb[:].rearrange("p (two one) -> p two one", one=1),
            in_=d_vidx.rearrange("two p one -> p two one"))
        for gi in range(2):
            vt = const.tile([128, P * HID + 128], BF16, tag=f"v{4 + gi}",
                            name=f"vg{gi}")
            nc.vector.memset(vt[:, P * HID:], 0.0)
            nc.gpsimd.indirect_dma_start(
                out=vt[:, :P * HID], out_offset=None,
                in_=d_vb[:, :],
                in_offset=bass.IndirectOffsetOnAxis(
                    ap=idx_sb[:, gi:gi + 1], axis=0))
            v_sb.append(vt)

        # recip workspace
        rec2 = const.tile([1, 2 * TQ], F32, tag="rec2")
        rscr = const.tile([1, 2 * TQ], F32, tag="rscr")

        # attention outputs (divided), one tile per head-pair: [96, 2*TQ]
        apair = []
        for h2 in range(H // 2):
            apair.append(attnp.tile([DH, 2 * TQ], F32, tag=f"apair{h2}",
                                    name=f"apair{h2}"))

        # ================= attention main loop =================
        for hg in range(HG):
            # per-hg psum accumulators (live across st loop)
            numer = [psum1.tile([128, 2 * TQ], F32, space="PSUM",
                                tag=f"numer_{i}",
                                name=f"numer{hg}_{i}") for i in range(2)]
            den_ps = [psum1.tile([1, 2 * TQ], F32, space="PSUM",
                                 tag=f"den_{i}", name=f"den{hg}_{i}")
                      for i in range(2)]
            for st in range(NST):
                scores = psum.tile([128, 4 * TQ], F32, space="PSUM",
                                   tag="scores")
                # bias preload into PSUM (identity matmul, fp32r full rate)
                bt = biasp.tile([128, 4 * TQ], F16, tag="bias")
                from contextlib import nullcontext
                with (tc.high_priority() if (hg == 0 and st < 2)
                      else nullcontext()):
                    nc.sync.dma_start(
                        out=bt[:].rearrange("p (i t) -> p i t", i=4),
                        in_=d_bias[st * 128:(st + 1) * 128,
                                   hg * 4:hg * 4 + 4, :])
                for half in range(2):
                    nc.tensor.matmul(
                        out=scores[:, half * 512:(half + 1) * 512],
                        lhsT=ident[:],
                        rhs=bt[:, half * 512:(half + 1) * 512],
                        start=True, stop=False)
                # QK accumulate on top (i=1,3 close their banks)
                for i in range(4):
                    h = hg * 4 + i
                    nc.tensor.matmul(
                        out=scores[:, i * TQ:(i + 1) * TQ],
                        lhsT=kTe[:, h * S + st * 128:h * S + (st + 1) * 128],
                        rhs=qT[:, h * TQ:(h + 1) * TQ],
                        start=False, stop=(i % 2 == 1))
                # m = (scores + 20) * law    [one fat DVE pass]
                m = work.tile([128, 4 * TQ], F32, tag="m")
                law_st = law[:, st * TQ:(st + 1) * TQ]
                nc.vector.scalar_tensor_tensor(
                    out=m[:].rearrange("p (i t) -> p i t", i=4),
                    in0=scores[:].rearrange("p (i t) -> p i t", i=4),
                    scalar=SMOOTH, in1=law_st.unsqueeze(1).to_broadcast([128, 4, TQ]),
                    op0=ALU.add, op1=ALU.mult)
                # e = exp(m - 20) -> bf16
                e = work.tile([128, 4 * TQ], BF16, tag="e")
                nc.scalar.activation(e[:], m[:], AF.Exp, bias=neg20[:], scale=1.0)
                # g = e * law (bf16)
                g = work.tile([128, 4 * TQ], BF16, tag="g")
                lawb_st = lawb[:, st * TQ:(st + 1) * TQ]
                geng = nc.gpsimd if st % 2 == 0 else nc.vector
                geng.tensor_tensor(
                    out=g[:].rearrange("p (i t) -> p i t", i=4),
                    in0=e[:].rearrange("p (i t) -> p i t", i=4),
                    in1=lawb_st.unsqueeze(1).to_broadcast([128, 4, TQ]),
                    op=ALU.mult)
                # denominators: ones.T @ e -> [1, 512] per head-pair
                for j in range(2):
                    nc.tensor.matmul(
                        out=den_ps[j][0:1, :],
                        lhsT=ones_b[:],
                        rhs=e[:, j * 512:(j + 1) * 512],
                        start=(st == 0), stop=(st == NST - 1))
                # numerators: v_ext.T @ g -> [96, TQ] per head
                for i in range(4):
                    h = hg * 4 + i
                    nc.tensor.matmul(
                        out=numer[i // 2][:, (i % 2) * TQ:(i % 2 + 1) * TQ],
                        lhsT=v_sb[st][:, h * DH:h * DH + 128],
                        rhs=g[:, i * TQ:(i + 1) * TQ],
                        start=(st == 0 and i % 2 == 0),
                        stop=(st == NST - 1 and i % 2 == 1))
            # drain: free numer psum fast with plain copies; recip rows
            # bounce to DRAM off the critical path
            h0 = hg * 4
            for i in range(2):
                nc.scalar.copy(apair[hg * 2 + i][:], numer[i][:96, :])
                nc.vector.reciprocal_approx_fast(
                    out=rec2[:, :], in_=den_ps[i][0:1, :])
                nc.sync.dma_start(
                    out=d_rec[h0 + 2 * i:h0 + 2 * i + 2, :],
                    in_=rec2[:, :].rearrange("one (j t) -> one j t", j=2))

        # ============ remap heads -> channel-major tiles ============
        # attn_raw_ct[ci] : [128 (c%128), P*TQ], c = h*32+dd, free = (p, t)
        attn_rawct = []
        rcp_ct = []
        attn_ct = []
        for ci in range(4):
            raw = attnp.tile([128, P * TQ], F32, tag=f"raw{ci}", name=f"raw{ci}")
            attn_rawct.append(raw)
            rcp = attnp.tile([128, P * TQ], F32, tag=f"rcp{ci}", name=f"rcp{ci}")
            rcp_ct.append(rcp)
            act = attnp.tile([128, P * TQ], F16, tag=f"act{ci}", name=f"act{ci}")
            attn_ct.append(act)
        for h in range(H):
            ci, r0 = h // 4, (h % 4) * 32
            for p in range(P):
                eng = nc.sync if (h + p) % 2 == 0 else nc.scalar
                eng.dma_start(
                    out=attn_rawct[ci][r0:r0 + 32, p * TQ:(p + 1) * TQ],
                    in_=apair[h // 2][p * 32:p * 32 + 32,
                                      (h % 2) * TQ:(h % 2 + 1) * TQ])
            # recip broadcast rows for this head: [32, (p, t)]
            nc.gpsimd.dma_start(
                out=rcp_ct[ci][r0:r0 + 32, :].rearrange(
                    "dd (p t) -> dd p t", p=P),
                in_=d_rec[h:h + 1, :].unsqueeze(1).to_broadcast((32, P, TQ)))
        # divide (fp16 out for the out_proj lhsT)
        for ci in range(4):
            nc.vector.tensor_tensor(
                out=attn_ct[ci][:], in0=attn_rawct[ci][:],
                in1=rcp_ct[ci][:], op=ALU.mult)

        # ================= equivariant LN =================
        sqp = work.tile([128, P * TQ], F32R, tag="sq")
        ssq_a = psum1.tile([1, 512], F32, space="PSUM", tag="den_0")
        ssq_b = psum1.tile([1, TQ], F32, space="PSUM", tag="den_1")
        for ci in range(4):
            aci = attn_ct[ci][:]
            nc.vector.tensor_tensor(out=sqp[:], in0=aci, in1=aci, op=ALU.mult)
            nc.tensor.matmul(out=ssq_a[0:1, :], lhsT=ones_f[:].bitcast(F32R),
                             rhs=sqp[:, 0:512],
                             start=(ci == 0), stop=(ci == 3))
            nc.tensor.matmul(out=ssq_b[0:1, :], lhsT=ones_f[:].bitcast(F32R),
                             rhs=sqp[:, 512:768],
                             start=(ci == 0), stop=(ci == 3))
        # fold p-blocks: y[t] = ssq(p0)+ssq(p1)+ssq(p2)
        yrow = const.tile([1, TQ], F32, tag="yrow")
        nc.vector.tensor_copy(yrow[:], ssq_a[0:1, 0:TQ])
        nc.vector.tensor_tensor(out=yrow[:], in0=yrow[:],
                                in1=ssq_a[0:1, TQ:2 * TQ], op=ALU.add)
        nc.vector.tensor_tensor(out=yrow[:], in0=yrow[:], in1=ssq_b[0:1, :],
                                op=ALU.add)
        # inorm = rsqrt(y/512 + eps), then one Newton step
        # r' = r*(1.5 - 0.5*a*r^2) with a = y/512+eps
        arow = const.tile([1, TQ], F32, tag="arow")
        nc.vector.tensor_scalar(
            out=arow[:], in0=yrow[:], scalar1=1.0 / HID, scalar2=EPS,
            op0=ALU.mult, op1=ALU.add)
        rcpa = const.tile([1, TQ], F32, tag="rcpa")
        nc.vector.reciprocal_approx_fast(out=rcpa[:], in_=arow[:])
        r0t = const.tile([1, TQ], F32, tag="r0t")
        nc.scalar.activation(r0t[:], rcpa[:], AF.Sqrt, bias=0.0, scale=1.0)
        tmp = const.tile([1, TQ], F32, tag="tmpn")
        nc.vector.tensor_tensor(out=tmp[:], in0=r0t[:], in1=r0t[:], op=ALU.mult)
        nc.vector.tensor_tensor(out=tmp[:], in0=tmp[:], in1=arow[:], op=ALU.mult)
        nc.vector.tensor_scalar(
            out=tmp[:], in0=tmp[:], scalar1=-0.5, scalar2=1.5,
            op0=ALU.mult, op1=ALU.add)
        inorm = const.tile([1, TQ], F32R, tag="inorm")
        nc.vector.tensor_tensor(out=inorm[:], in0=r0t[:], in1=tmp[:],
                                op=ALU.mult)
        # inorm as columns [128,1] per t-half (bounce through DRAM)
        icol = const.tile([128, 2], F32, tag="icol")
        nc.sync.dma_start(out=d_ino.rearrange("(one t) -> one t", one=1),
                          in_=inorm[:, :].bitcast(F32))
        for th in range(2):
            nc.sync.dma_start(
                out=icol[:, th:th + 1],
                in_=d_ino[th * 128:(th + 1) * 128]
                    .rearrange("(p one) -> p one", one=1))

        # ================= out_proj =================
        nc.sync.dma_start(
            out=wT[:].rearrange("p (ci o) -> p ci o", ci=4),
            in_=d_wT.rearrange("(ci p) o -> p ci o", p=128))
        for k in range(6):          # tp-tiles: p = k//2, t-half = k%2
            op = psum.tile([128, HID], F32, space="PSUM", tag="scores")
            for ci in range(4):
                nc.tensor.matmul(
                    out=op[:, :],
                    lhsT=attn_ct[ci][:, k * 128:(k + 1) * 128],
                    rhs=wT[:, ci * HID:(ci + 1) * HID],
                    start=(ci == 0), stop=(ci == 3))
            ot = work.tile([128, HID], F32, tag="osb")
            nc.vector.tensor_scalar(
                out=ot[:], in0=op[:, :], scalar1=icol[:, k % 2:k % 2 + 1],
                scalar2=None, op0=ALU.mult)
            nc.sync.dma_start(
                out=d_out[(k % 2) * 128:(k % 2) * 128 + 128, k // 2, :],
                in_=ot[:])


def _host_prep(q, k, v, attn_bias, local_attention_weight, out_proj_w,
               ln_weight, outcell_index):
    """Pure layout marshalling on host -> per-core input dicts."""
    q = np.asarray(q, np.float32)
    k = np.asarray(k, np.float32)
    v = np.asarray(v, np.float32)
    attn_bias = np.asarray(attn_bias, np.float32)
    law = np.asarray(local_attention_weight, np.float32)
    out_proj_w = np.asarray(out_proj_w, np.float32)
    ln_weight = np.asarray(ln_weight, np.float32)
    idx = np.asarray(outcell_index).astype(np.int64)

    # (B,T,P,HID) -> (B, 96, H, T) with row j = p*32+dd
    def to_dT(x):
        return np.ascontiguousarray(
            x.reshape(B, T, P, H, D).transpose(0, 2, 4, 3, 1)
        ).reshape(B, P * D, H, T)

    qT = to_dT(q) * np.float32(SCALING)
    kT = to_dT(k)
    # K PBC expansion along token axis (gather columns)
    kTe = np.concatenate(
        [kT, np.take_along_axis(
            kT, idx[:, None, None, :].astype(np.int64), axis=3)], axis=3)
    biasT = np.ascontiguousarray(
        attn_bias.transpose(0, 3, 1, 2)).astype(np.float16)       # (B,S,H,T)
    lawT = np.ascontiguousarray(law.transpose(0, 2, 1))            # (B,S,T)
    lawTb = lawT.astype(ml_dtypes.bfloat16)
    # head-major V columns: (B, T, (h, p, dd)) so each head is contiguous
    vb = np.ascontiguousarray(
        v.reshape(B, T, P, H, D).transpose(0, 1, 3, 2, 4)
    ).reshape(B, T, P * HID).astype(ml_dtypes.bfloat16)
    wT = np.ascontiguousarray(out_proj_w.T) * ln_weight[:, None]   # (c,o)
    wT = np.ascontiguousarray(wT, np.float32).astype(np.float16)
    vidx = idx.astype(np.int32).reshape(B, 2, 128, 1)

    in_maps = []
    for c in range(NCORES):
        b, th = c // 2, c % 2
        t0 = th * TQ
        lawc = np.ascontiguousarray(lawT[b, :, t0:t0 + TQ])
        lawc = np.ascontiguousarray(
            lawc.reshape(NST, 128, TQ).transpose(1, 0, 2)).reshape(128, NST * TQ)
        lawcb = np.ascontiguousarray(lawTb[b, :, t0:t0 + TQ])
        lawcb = np.ascontiguousarray(
            lawcb.reshape(NST, 128, TQ).transpose(1, 0, 2)).reshape(128, NST * TQ)
        in_maps.append(dict(
            biasT=np.ascontiguousarray(biasT[b, :, :, t0:t0 + TQ]),
            lawT=lawc,
            lawTb=lawcb,
            qT=np.ascontiguousarray(
                qT[b, :, :, t0:t0 + TQ]).reshape(DH, H * TQ)
                .astype(ml_dtypes.bfloat16),
            kTe=np.ascontiguousarray(kTe[b]).reshape(DH, H * S)
                .astype(ml_dtypes.bfloat16),
            vb=np.ascontiguousarray(vb[b]),
            vidx=np.ascontiguousarray(vidx[b]),
            wT=wT,
            ident=np.eye(128, dtype=np.float16),
        ))
    return in_maps


def kernel(**inputs):
    global _CACHED_NC
    if _CACHED_NC is None:
        _CACHED_NC = build_nc()
    nc = _CACHED_NC
    in_maps = _host_prep(
        inputs["q"], inputs["k"], inputs["v"], inputs["attn_bias"],
        inputs["local_attention_weight"], inputs["out_proj_w"],
        inputs["ln_weight"], inputs["outcell_index"])
    res = run_bass_kernel_spmd(nc, in_maps, core_ids=list(range(NCORES)))
    out = np.empty((B, T, P, HID), np.float32)
    for c in range(NCORES):
        b, th = c // 2, c % 2
        out[b, th * TQ:(th + 1) * TQ] = res.results[c]["out"]
    return out


# revision 37
# speedup vs baseline: 1.1498x; 1.0300x over previous
"""Trainium2 Bass kernel for EquivariantAttention (sparse_attention).

Full (unsharded) inputs in, full output out. Internally shards over the 8
NeuronCores as (batch, T-half): core c handles batch b = c // 2, query rows
t0 = (c % 2) * 256 .. t0+256.  Every core runs the identical SPMD program on
its own input slices; there is no cross-core communication (LN and out_proj
are row-local in (b, t)).

Device-side per core:
  scores_T[s,t] = bias_T + (k_T.T @ q_T)          (bias preloaded into PSUM
                                                   via identity matmul, QK
                                                   accumulates on top, fp32r)
  m = (scores_T + 20) * law_T                     (one DVE pass, fp32)
  e = exp(m - 20)  -> bf16                        (ACT, free affine bias)
  g = e * law_T                                   (DVE, bf16)
  denom[t] = sum_s e                              (ones-column matmul)
  numer[j,t] = sum_s v_ext[s,j] * g[s,t]          (bf16 matmul; V PBC rows
                                                   gathered on device by
                                                   indirect DMA)
  attn = numer * recip(denom)                     (DVE)
  inorm = rsqrt(SSQ/512 + 1e-3)                   (SSQ via ones-matmul of
                                                   attn^2, ACT rsqrt +
                                                   1 Newton step)
  out = (attn @ (out_proj_w * ln_w).T) * inorm    (fp32r matmuls, inorm
                                                   folded into PSUM->SBUF
                                                   copy-out)
"""

import numpy as np
import ml_dtypes

import concourse.bass as bass
import concourse.bacc as bacc
import concourse.tile as tile
from concourse import mybir
from concourse.bass_utils import run_bass_kernel_spmd

# Problem constants (hardcoded per contract)
B, T, P, HID = 4, 512, 3, 512
H, D = 16, 32
EXP = 256
S = T + EXP            # 768
SCALING = (D / 3.0) ** 0.5 / D
SMOOTH = 20.0
EPS = 1e-3

NCORES = 8
TQ = T // 2            # 256 query rows per core
DH = P * D             # 96 head dim
NST = S // 128         # 6 s-tiles of 128
HG = 4                 # head groups of 4 heads

F32 = mybir.dt.float32
F16 = mybir.dt.float16
F32R = mybir.dt.float32r
BF16 = mybir.dt.bfloat16
I32 = mybir.dt.int32
AF = mybir.ActivationFunctionType
ALU = mybir.AluOpType

_CACHED_NC = None


def r(ap):
    """bitcast f32 AP -> f32r for full-rate PE."""
    return ap.bitcast(F32R)


def build_nc():
    nc = bacc.Bacc("TRN2", target_bir_lowering=False, debug=False)

    # ---- DRAM I/O (per-core shapes) ----
    d_bias = nc.dram_tensor("biasT", [S, H, TQ], F16, kind="ExternalInput").ap()
    d_law = nc.dram_tensor("lawT", [128, NST * TQ], F32, kind="ExternalInput").ap()
    d_lawb = nc.dram_tensor("lawTb", [128, NST * TQ], BF16, kind="ExternalInput").ap()
    d_qT = nc.dram_tensor("qT", [DH, H * TQ], BF16, kind="ExternalInput").ap()
    d_kTe = nc.dram_tensor("kTe", [DH, H * S], BF16, kind="ExternalInput").ap()
    d_vb = nc.dram_tensor("vb", [T, P * HID], BF16, kind="ExternalInput").ap()
    d_vidx = nc.dram_tensor("vidx", [2, 128, 1], I32, kind="ExternalInput").ap()
    d_wT = nc.dram_tensor("wT", [HID, HID], F16, kind="ExternalInput").ap()
    d_id = nc.dram_tensor("ident", [128, 128], F16, kind="ExternalInput").ap()
    d_out = nc.dram_tensor("out", [TQ, P, HID], F32, kind="ExternalOutput").ap()
    d_rec = nc.dram_tensor("rec_scratch", [16, TQ], F32).ap()
    d_ino = nc.dram_tensor("inorm_scratch", [TQ], F32).ap()

    with tile.TileContext(nc) as tc:
        build_kernel(tc, d_bias, d_law, d_lawb, d_qT, d_kTe, d_vb, d_vidx,
                     d_wT, d_id, d_out, d_rec, d_ino)
    nc.compile()
    return nc


def build_kernel(tc, d_bias, d_law, d_lawb, d_qT, d_kTe, d_vb, d_vidx,
                 d_wT, d_id, d_out, d_rec, d_ino):
    nc = tc.nc
    from contextlib import ExitStack
    ctx = ExitStack()
    with ctx:
        const = ctx.enter_context(tc.tile_pool(name="const", bufs=1))
        big = ctx.enter_context(tc.tile_pool(name="big", bufs=1))
        biasp = ctx.enter_context(tc.tile_pool(name="biasp", bufs=4))
        work = ctx.enter_context(tc.tile_pool(name="work", bufs=2))
        attnp = ctx.enter_context(tc.tile_pool(name="attnp", bufs=1))
        psum = ctx.enter_context(tc.tile_pool(name="psum", bufs=2, space="PSUM"))
        psum1 = ctx.enter_context(tc.tile_pool(name="psum1", bufs=1, space="PSUM"))

        # ---- constants ----
        ident = const.tile([128, 128], F16, tag="ident")
        nc.sync.dma_start(out=ident[:], in_=d_id)
        ones_b = const.tile([128, 1], BF16, tag="ones_b")
        nc.vector.memset(ones_b[:], 1.0)
        ones_f = const.tile([128, 1], F32, tag="ones_f")
        nc.vector.memset(ones_f[:], 1.0)
        neg20 = const.tile([128, 1], F32, tag="neg20")
        nc.vector.memset(neg20[:], -SMOOTH)
        ones_r = const.tile([1, 1], F32R, tag="ones_r")
        nc.vector.tensor_copy(ones_r[:], ones_f[0:1, 0:1])

        # ---- resident loads ----
        law = const.tile([128, NST * TQ], F32, tag="law")      # (s%128,(st,t))
        lawb = const.tile([128, NST * TQ], BF16, tag="lawb")
        qT = const.tile([DH, H * TQ], BF16, tag="qT")
        kTe = big.tile([DH, H * S], BF16, tag="kTe")
        # need-ordered: first head-group's k/q first, law early (stt dep)
        with tc.high_priority():
            nc.sync.dma_start(out=law[:], in_=d_law)
            nc.sync.dma_start(out=kTe[:, :4 * S], in_=d_kTe[:, :4 * S])
            nc.sync.dma_start(out=qT[:, :4 * TQ], in_=d_qT[:, :4 * TQ])
        nc.sync.dma_start(out=qT[:, 4 * TQ:8 * TQ],
                          in_=d_qT[:, 4 * TQ:8 * TQ])
        for c in range(1, 4):
            nc.sync.dma_start(
                out=kTe[:, c * 4 * S:(c + 1) * 4 * S],
                in_=d_kTe[:, c * 4 * S:(c + 1) * 4 * S])
            if c == 1:
                nc.sync.dma_start(out=lawb[:], in_=d_lawb)
                nc.sync.dma_start(out=qT[:, 8 * TQ:], in_=d_qT[:, 8 * TQ:])
        wT = const.tile([128, 4 * HID], F16, tag="wT")          # (c%128,(ci,o))

        # V tiles: 4 direct + 2 gathered (PBC expansion), bf16
        v_sb = []
        for st in range(4):
            vt = const.tile([128, P * HID + 128], BF16, tag=f"v{st}",
                            name=f"v{st}")
            nc.vector.memset(vt[:, P * HID:], 0.0)
            nc.sync.dma_start(out=vt[:, :P * HID],
                              in_=d_vb[st * 128:(st + 1) * 128, :])
            v_sb.append(vt)
        idx_sb = const.tile([128, 2], I32, tag="idx")
        nc.gpsimd.dma_start(
            out=idx_sb[:].rearrange("p (two one) -> p two one", one=1),
            in_=d_vidx.rearrange("two p one -> p two one"))
        for gi in range(2):
            vt = const.tile([128, P * HID + 128], BF16, tag=f"v{4 + gi}",
                            name=f"vg{gi}")
            nc.vector.memset(vt[:, P * HID:], 0.0)
            nc.gpsimd.indirect_dma_start(
                out=vt[:, :P * HID], out_offset=None,
                in_=d_vb[:, :],
                in_offset=bass.IndirectOffsetOnAxis(
                    ap=idx_sb[:, gi:gi + 1], axis=0))
            v_sb.append(vt)

        # recip workspace
        rec2 = const.tile([1, 2 * TQ], F32, tag="rec2")
        rscr = const.tile([1, 2 * TQ], F32, tag="rscr")

        # attention outputs (divided), one tile per head-pair: [96, 2*TQ]
        apair = []
        for h2 in range(H // 2):
            apair.append(attnp.tile([DH, 2 * TQ], F32, tag=f"apair{h2}",
                                    name=f"apair{h2}"))

        # ================= attention main loop =================
        for hg in range(HG):
            # per-hg psum accumulators (live across st loop)
            numer = [psum1.tile([128, 2 * TQ], F32, space="PSUM",
                                tag=f"numer_{i}",
                                name=f"numer{hg}_{i}") for i in range(2)]
            den_ps = [psum1.tile([1, 2 * TQ], F32, space="PSUM",
                                 tag=f"den_{i}", name=f"den{hg}_{i}")
                      for i in range(2)]
            for st in range(NST):
                scores = psum.tile([128, 4 * TQ], F32, space="PSUM",
                                   tag="scores")
                # bias preload into PSUM (identity matmul, fp32r full rate)
                bt = biasp.tile([128, 4 * TQ], F16, tag="bias")
                from contextlib import nullcontext
                with (tc.high_priority() if (hg == 0 and st < 2)
                      else nullcontext()):
                    nc.sync.dma_start(
                        out=bt[:].rearrange("p (i t) -> p i t", i=4),
                        in_=d_bias[st * 128:(st + 1) * 128,
                                   hg * 4:hg * 4 + 4, :])
                for half in range(2):
                    nc.tensor.matmul(
                        out=scores[:, half * 512:(half + 1) * 512],
                        lhsT=ident[:],
                        rhs=bt[:, half * 512:(half + 1) * 512],
                        start=True, stop=False)
                # QK accumulate on top (i=1,3 close their banks)
                for i in range(4):
                    h = hg * 4 + i
                    nc.tensor.matmul(
                        out=scores[:, i * TQ:(i + 1) * TQ],
                        lhsT=kTe[:, h * S + st * 128:h * S + (st + 1) * 128],
                        rhs=qT[:, h * TQ:(h + 1) * TQ],
                        start=False, stop=(i % 2 == 1))
                # m = (scores + 20) * law    [one fat DVE pass]
                m = work.tile([128, 4 * TQ], F32, tag="m")
                law_st = law[:, st * TQ:(st + 1) * TQ]
                nc.vector.scalar_tensor_tensor(
                    out=m[:].rearrange("p (i t) -> p i t", i=4),
                    in0=scores[:].rearrange("p (i t) -> p i t", i=4),
                    scalar=SMOOTH, in1=law_st.unsqueeze(1).to_broadcast([128, 4, TQ]),
                    op0=ALU.add, op1=ALU.mult)
                # e = exp(m - 20) -> bf16
                e = work.tile([128, 4 * TQ], BF16, tag="e")
                nc.scalar.activation(e[:], m[:], AF.Exp, bias=neg20[:], scale=1.0)
                # g = e * law (bf16)
                g = work.tile([128, 4 * TQ], BF16, tag="g")
                lawb_st = lawb[:, st * TQ:(st + 1) * TQ]
                geng = nc.gpsimd if st % 2 == 0 else nc.vector
                geng.tensor_tensor(
                    out=g[:].rearrange("p (i t) -> p i t", i=4),
                    in0=e[:].rearrange("p (i t) -> p i t", i=4),
                    in1=lawb_st.unsqueeze(1).to_broadcast([128, 4, TQ]),
                    op=ALU.mult)
                # denominators: ones.T @ e -> [1, 512] per head-pair
                for j in range(2):
                    nc.tensor.matmul(
                        out=den_ps[j][0:1, :],
                        lhsT=ones_b[:],
                        rhs=e[:, j * 512:(j + 1) * 512],
                        start=(st == 0), stop=(st == NST - 1))
                # numerators: v_ext.T @ g -> [96, TQ] per head
                for i in range(4):
                    h = hg * 4 + i
                    nc.tensor.matmul(
                        out=numer[i // 2][:, (i % 2) * TQ:(i % 2 + 1) * TQ],
                        lhsT=v_sb[st][:, h * DH:h * DH + 128],
                        rhs=g[:, i * TQ:(i + 1) * TQ],
                        start=(st == 0 and i % 2 == 0),
                        stop=(st == NST - 1 and i % 2 == 1))
            # drain: free numer psum fast with plain copies; recip rows
            # bounce to DRAM off the critical path
            h0 = hg * 4
            for i in range(2):
                nc.scalar.copy(apair[hg * 2 + i][:], numer[i][:96, :])
                nc.vector.reciprocal_approx_fast(
                    out=rec2[:, :], in_=den_ps[i][0:1, :])
                nc.sync.dma_start(
                    out=d_rec[h0 + 2 * i:h0 + 2 * i + 2, :],
                    in_=rec2[:, :].rearrange("one (j t) -> one j t", j=2))

        # ============ remap heads -> channel-major tiles ============
        # attn_raw_ct[ci] : [128 (c%128), P*TQ], c = h*32+dd, free = (p, t)
        attn_rawct = []
        rcp_ct = []
        attn_ct = []
        for ci in range(4):
            raw = attnp.tile([128, P * TQ], F32, tag=f"raw{ci}", name=f"raw{ci}")
            attn_rawct.append(raw)
            rcp = attnp.tile([128, P * TQ], F32, tag=f"rcp{ci}", name=f"rcp{ci}")
            rcp_ct.append(rcp)
            act = attnp.tile([128, P * TQ], F16, tag=f"act{ci}", name=f"act{ci}")
            attn_ct.append(act)
        for h in range(H):
            ci, r0 = h // 4, (h % 4) * 32
            for p in range(P):
                eng = nc.sync if (h + p) % 2 == 0 else nc.scalar
                eng.dma_start(
                    out=attn_rawct[ci][r0:r0 + 32, p * TQ:(p + 1) * TQ],
                    in_=apair[h // 2][p * 32:p * 32 + 32,
                                      (h % 2) * TQ:(h % 2 + 1) * TQ])
            # recip broadcast rows for this head: [32, (p, t)]
            nc.gpsimd.dma_start(
                out=rcp_ct[ci][r0:r0 + 32, :].rearrange(
                    "dd (p t) -> dd p t", p=P),
                in_=d_rec[h:h + 1, :].unsqueeze(1).to_broadcast((32, P, TQ)))
        # divide (fp16 out for the out_proj lhsT)
        for ci in range(4):
            nc.vector.tensor_tensor(
                out=attn_ct[ci][:], in0=attn_rawct[ci][:],
                in1=rcp_ct[ci][:], op=ALU.mult)

        # ================= equivariant LN =================
        sqp = work.tile([128, P * TQ], F32R, tag="sq")
        ssq_a = psum1.tile([1, 512], F32, space="PSUM", tag="den_0")
        ssq_b = psum1.tile([1, TQ], F32, space="PSUM", tag="den_1")
        for ci in range(4):
            aci = attn_ct[ci][:]
            nc.vector.tensor_tensor(out=sqp[:], in0=aci, in1=aci, op=ALU.mult)
            nc.tensor.matmul(out=ssq_a[0:1, :], lhsT=ones_f[:].bitcast(F32R),
                             rhs=sqp[:, 0:512],
                             start=(ci == 0), stop=(ci == 3))
            nc.tensor.matmul(out=ssq_b[0:1, :], lhsT=ones_f[:].bitcast(F32R),
                             rhs=sqp[:, 512:768],
                             start=(ci == 0), stop=(ci == 3))
        # fold p-blocks: y[t] = ssq(p0)+ssq(p1)+ssq(p2)
        yrow = const.tile([1, TQ], F32, tag="yrow")
        nc.vector.tensor_copy(yrow[:], ssq_a[0:1, 0:TQ])
        nc.vector.tensor_tensor(out=yrow[:], in0=yrow[:],
                                in1=ssq_a[0:1, TQ:2 * TQ], op=ALU.add)
        nc.vector.tensor_tensor(out=yrow[:], in0=yrow[:], in1=ssq_b[0:1, :],
                                op=ALU.add)
        # inorm = rsqrt(y/512 + eps), then one Newton step
        # r' = r*(1.5 - 0.5*a*r^2) with a = y/512+eps
        arow = const.tile([1, TQ], F32, tag="arow")
        nc.vector.tensor_scalar(
            out=arow[:], in0=yrow[:], scalar1=1.0 / HID, scalar2=EPS,
            op0=ALU.mult, op1=ALU.add)
        rcpa = const.tile([1, TQ], F32, tag="rcpa")
        nc.vector.reciprocal_approx_fast(out=rcpa[:], in_=arow[:])
        r0t = const.tile([1, TQ], F32, tag="r0t")
        nc.scalar.activation(r0t[:], rcpa[:], AF.Sqrt, bias=0.0, scale=1.0)
        tmp = const.tile([1, TQ], F32, tag="tmpn")
        nc.vector.tensor_tensor(out=tmp[:], in0=r0t[:], in1=r0t[:], op=ALU.mult)
        nc.vector.tensor_tensor(out=tmp[:], in0=tmp[:], in1=arow[:], op=ALU.mult)
        nc.vector.tensor_scalar(
            out=tmp[:], in0=tmp[:], scalar1=-0.5, scalar2=1.5,
            op0=ALU.mult, op1=ALU.add)
        inorm = const.tile([1, TQ], F32R, tag="inorm")
        nc.vector.tensor_tensor(out=inorm[:], in0=r0t[:], in1=tmp[:],
                                op=ALU.mult)
        # inorm as columns [128,1] per t-half (bounce through DRAM)
        icol = const.tile([128, 2], F32, tag="icol")
        nc.sync.dma_start(out=d_ino.rearrange("(one t) -> one t", one=1),
                          in_=inorm[:, :].bitcast(F32))
        for th in range(2):
            nc.sync.dma_start(
                out=icol[:, th:th + 1],
                in_=d_ino[th * 128:(th + 1) * 128]
                    .rearrange("(p one) -> p one", one=1))

        # ================= out_proj =================
        nc.sync.dma_start(
            out=wT[:].rearrange("p (ci o) -> p ci o", ci=4),
            in_=d_wT.rearrange("(ci p) o -> p ci o", p=128))
        for k in range(6):          # tp-tiles: p = k//2, t-half = k%2
            op = psum.tile([128, HID], F32, space="PSUM", tag="scores")
            for ci in range(4):
                nc.tensor.matmul(
                    out=op[:, :],
                    lhsT=attn_ct[ci][:, k * 128:(k + 1) * 128],
                    rhs=wT[:, ci * HID:(ci + 1) * HID],
                    start=(ci == 0), stop=(ci == 3))
            ot = work.tile([128, HID], F32, tag="osb")
            nc.vector.tensor_scalar(
                out=ot[:], in0=op[:, :], scalar1=icol[:, k % 2:k % 2 + 1],
                scalar2=None, op0=ALU.mult)
            nc.sync.dma_start(
                out=d_out[(k % 2) * 128:(k % 2) * 128 + 128, k // 2, :],
                in_=ot[:])


def _host_prep(q, k, v, attn_bias, local_attention_weight, out_proj_w,
               ln_weight, outcell_index):
    """Pure layout marshalling on host -> per-core input dicts."""
    q = np.asarray(q, np.float32)
    k = np.asarray(k, np.float32)
    v = np.asarray(v, np.float32)
    attn_bias = np.asarray(attn_bias, np.float32)
    law = np.asarray(local_attention_weight, np.float32)
    out_proj_w = np.asarray(out_proj_w, np.float32)
    ln_weight = np.asarray(ln_weight, np.float32)
    idx = np.asarray(outcell_index).astype(np.int64)

    # (B,T,P,HID) -> (B, 96, H, T) with row j = p*32+dd
    def to_dT(x):
        return np.ascontiguousarray(
            x.reshape(B, T, P, H, D).transpose(0, 2, 4, 3, 1)
        ).reshape(B, P * D, H, T)

    qT = to_dT(q) * np.float32(SCALING)
    kT = to_dT(k)
    # K PBC expansion along token axis (gather columns)
    kTe = np.concatenate(
        [kT, np.take_along_axis(
            kT, idx[:, None, None, :].astype(np.int64), axis=3)], axis=3)
    biasT = np.ascontiguousarray(
        attn_bias.transpose(0, 3, 1, 2)).astype(np.float16)       # (B,S,H,T)
    lawT = np.ascontiguousarray(law.transpose(0, 2, 1))            # (B,S,T)
    lawTb = lawT.astype(ml_dtypes.bfloat16)
    # head-major V columns: (B, T, (h, p, dd)) so each head is contiguous
    vb = np.ascontiguousarray(
        v.reshape(B, T, P, H, D).transpose(0, 1, 3, 2, 4)
    ).reshape(B, T, P * HID).astype(ml_dtypes.bfloat16)
    wT = np.ascontiguousarray(out_proj_w.T) * ln_weight[:, None]   # (c,o)
    wT = np.ascontiguousarray(wT, np.float32).astype(np.float16)
    vidx = idx.astype(np.int32).reshape(B, 2, 128, 1)

    in_maps = []
    for c in range(NCORES):
        b, th = c // 2, c % 2
        t0 = th * TQ
        lawc = np.ascontiguousarray(lawT[b, :, t0:t0 + TQ])
        lawc = np.ascontiguousarray(
            lawc.reshape(NST, 128, TQ).transpose(1, 0, 2)).reshape(128, NST * TQ)
        lawcb = np.ascontiguousarray(lawTb[b, :, t0:t0 + TQ])
        lawcb = np.ascontiguousarray(
            lawcb.reshape(NST, 128, TQ).transpose(1, 0, 2)).reshape(128, NST * TQ)
        in_maps.append(dict(
            biasT=np.ascontiguousarray(biasT[b, :, :, t0:t0 + TQ]),
            lawT=lawc,
            lawTb=lawcb,
            qT=np.ascontiguousarray(
                qT[b, :, :, t0:t0 + TQ]).reshape(DH, H * TQ)
                .astype(ml_dtypes.bfloat16),
            kTe=np.ascontiguousarray(kTe[b]).reshape(DH, H * S)
                .astype(ml_dtypes.bfloat16),
            vb=np.ascontiguousarray(vb[b]),
            vidx=np.ascontiguousarray(vidx[b]),
            wT=wT,
            ident=np.eye(128, dtype=np.float16),
        ))
    return in_maps


def kernel(**inputs):
    global _CACHED_NC
    if _CACHED_NC is None:
        _CACHED_NC = build_nc()
    nc = _CACHED_NC
    in_maps = _host_prep(
        inputs["q"], inputs["k"], inputs["v"], inputs["attn_bias"],
        inputs["local_attention_weight"], inputs["out_proj_w"],
        inputs["ln_weight"], inputs["outcell_index"])
    res = run_bass_kernel_spmd(nc, in_maps, core_ids=list(range(NCORES)))
    out = np.empty((B, T, P, HID), np.float32)
    for c in range(NCORES):
        b, th = c // 2, c % 2
        out[b, th * TQ:(th + 1) * TQ] = res.results[c]["out"]
    return out


# revision 38
# speedup vs baseline: 1.1737x; 1.0208x over previous
"""Trainium2 Bass kernel for EquivariantAttention (sparse_attention).

Full (unsharded) inputs in, full output out. Internally shards over the 8
NeuronCores as (batch, T-half): core c handles batch b = c // 2, query rows
t0 = (c % 2) * 256 .. t0+256.  Every core runs the identical SPMD program on
its own input slices; there is no cross-core communication (LN and out_proj
are row-local in (b, t)).

Device-side per core:
  scores_T[s,t] = bias_T + (k_T.T @ q_T)          (bias preloaded into PSUM
                                                   via identity matmul, QK
                                                   accumulates on top, fp32r)
  m = (scores_T + 20) * law_T                     (one DVE pass, fp32)
  e = exp(m - 20)  -> bf16                        (ACT, free affine bias)
  g = e * law_T                                   (DVE, bf16)
  denom[t] = sum_s e                              (ones-column matmul)
  numer[j,t] = sum_s v_ext[s,j] * g[s,t]          (bf16 matmul; V PBC rows
                                                   gathered on device by
                                                   indirect DMA)
  attn = numer * recip(denom)                     (DVE)
  inorm = rsqrt(SSQ/512 + 1e-3)                   (SSQ via ones-matmul of
                                                   attn^2, ACT rsqrt +
                                                   1 Newton step)
  out = (attn @ (out_proj_w * ln_w).T) * inorm    (fp32r matmuls, inorm
                                                   folded into PSUM->SBUF
                                                   copy-out)
"""

import numpy as np
import ml_dtypes

import concourse.bass as bass
import concourse.bacc as bacc
import concourse.tile as tile
from concourse import mybir
from concourse.bass_utils import run_bass_kernel_spmd

# Problem constants (hardcoded per contract)
B, T, P, HID = 4, 512, 3, 512
H, D = 16, 32
EXP = 256
S = T + EXP            # 768
SCALING = (D / 3.0) ** 0.5 / D
SMOOTH = 20.0
EPS = 1e-3

NCORES = 8
TQ = T // 2            # 256 query rows per core
DH = P * D             # 96 head dim
NST = S // 128         # 6 s-tiles of 128
HG = 4                 # head groups of 4 heads

F32 = mybir.dt.float32
F16 = mybir.dt.float16
F32R = mybir.dt.float32r
BF16 = mybir.dt.bfloat16
I32 = mybir.dt.int32
AF = mybir.ActivationFunctionType
ALU = mybir.AluOpType

_CACHED_NC = None


def r(ap):
    """bitcast f32 AP -> f32r for full-rate PE."""
    return ap.bitcast(F32R)


def build_nc():
    nc = bacc.Bacc("TRN2", target_bir_lowering=False, debug=False)

    # ---- DRAM I/O (per-core shapes) ----
    d_bias = nc.dram_tensor("biasT", [S, H, TQ], F16, kind="ExternalInput").ap()
    d_law = nc.dram_tensor("lawT", [128, NST * TQ], F32, kind="ExternalInput").ap()
    d_lawb = nc.dram_tensor("lawTb", [128, NST * TQ], BF16, kind="ExternalInput").ap()
    d_qT = nc.dram_tensor("qT", [DH, H * TQ], BF16, kind="ExternalInput").ap()
    d_kTe = nc.dram_tensor("kTe", [DH, H * S], BF16, kind="ExternalInput").ap()
    d_vb = nc.dram_tensor("vb", [T, P * HID], BF16, kind="ExternalInput").ap()
    d_vidx = nc.dram_tensor("vidx", [2, 128, 1], I32, kind="ExternalInput").ap()
    d_wT = nc.dram_tensor("wT", [HID, HID], F16, kind="ExternalInput").ap()
    d_id = nc.dram_tensor("ident", [128, 128], F16, kind="ExternalInput").ap()
    d_out = nc.dram_tensor("out", [TQ, P, HID], F32, kind="ExternalOutput").ap()
    d_rec = nc.dram_tensor("rec_scratch", [16, TQ], F32).ap()
    d_ino = nc.dram_tensor("inorm_scratch", [TQ], F32).ap()

    with tile.TileContext(nc) as tc:
        build_kernel(tc, d_bias, d_law, d_lawb, d_qT, d_kTe, d_vb, d_vidx,
                     d_wT, d_id, d_out, d_rec, d_ino)
    nc.compile()
    return nc


def build_kernel(tc, d_bias, d_law, d_lawb, d_qT, d_kTe, d_vb, d_vidx,
                 d_wT, d_id, d_out, d_rec, d_ino):
    nc = tc.nc
    from contextlib import ExitStack
    ctx = ExitStack()
    with ctx:
        const = ctx.enter_context(tc.tile_pool(name="const", bufs=1))
        big = ctx.enter_context(tc.tile_pool(name="big", bufs=1))
        biasp = ctx.enter_context(tc.tile_pool(name="biasp", bufs=6))
        work = ctx.enter_context(tc.tile_pool(name="work", bufs=2))
        attnp = ctx.enter_context(tc.tile_pool(name="attnp", bufs=1))
        psum = ctx.enter_context(tc.tile_pool(name="psum", bufs=2, space="PSUM"))
        psum1 = ctx.enter_context(tc.tile_pool(name="psum1", bufs=1, space="PSUM"))

        # ---- constants ----
        ident = const.tile([128, 128], F16, tag="ident")
        nc.sync.dma_start(out=ident[:], in_=d_id)
        ones_b = const.tile([128, 1], BF16, tag="ones_b")
        nc.vector.memset(ones_b[:], 1.0)
        ones_f = const.tile([128, 1], F32, tag="ones_f")
        nc.vector.memset(ones_f[:], 1.0)
        neg20 = const.tile([128, 1], F32, tag="neg20")
        nc.vector.memset(neg20[:], -SMOOTH)
        ones_r = const.tile([1, 1], F32R, tag="ones_r")
        nc.vector.tensor_copy(ones_r[:], ones_f[0:1, 0:1])

        # ---- resident loads ----
        law = const.tile([128, NST * TQ], F32, tag="law")      # (s%128,(st,t))
        lawb = const.tile([128, NST * TQ], BF16, tag="lawb")
        qT = const.tile([DH, H * TQ], BF16, tag="qT")
        kTe = big.tile([DH, H * S], BF16, tag="kTe")
        # need-ordered: first head-group's k/q first, law early (stt dep)
        with tc.high_priority():
            nc.sync.dma_start(out=law[:], in_=d_law)
            nc.sync.dma_start(out=kTe[:, :4 * S], in_=d_kTe[:, :4 * S])
            nc.sync.dma_start(out=qT[:, :4 * TQ], in_=d_qT[:, :4 * TQ])
        nc.sync.dma_start(out=qT[:, 4 * TQ:8 * TQ],
                          in_=d_qT[:, 4 * TQ:8 * TQ])
        for c in range(1, 4):
            nc.sync.dma_start(
                out=kTe[:, c * 4 * S:(c + 1) * 4 * S],
                in_=d_kTe[:, c * 4 * S:(c + 1) * 4 * S])
            if c == 1:
                nc.sync.dma_start(out=lawb[:], in_=d_lawb)
                nc.sync.dma_start(out=qT[:, 8 * TQ:], in_=d_qT[:, 8 * TQ:])
        wT = const.tile([128, 4 * HID], F16, tag="wT")          # (c%128,(ci,o))

        # V tiles: 4 direct + 2 gathered (PBC expansion), bf16
        v_sb = []
        for st in range(4):
            vt = const.tile([128, P * HID + 128], BF16, tag=f"v{st}",
                            name=f"v{st}")
            nc.vector.memset(vt[:, P * HID:], 0.0)
            nc.sync.dma_start(out=vt[:, :P * HID],
                              in_=d_vb[st * 128:(st + 1) * 128, :])
            v_sb.append(vt)
        idx_sb = const.tile([128, 2], I32, tag="idx")
        nc.gpsimd.dma_start(
            out=idx_sb[:].rearrange("p (two one) -> p two one", one=1),
            in_=d_vidx.rearrange("two p one -> p two one"))
        for gi in range(2):
            vt = const.tile([128, P * HID + 128], BF16, tag=f"v{4 + gi}",
                            name=f"vg{gi}")
            nc.vector.memset(vt[:, P * HID:], 0.0)
            nc.gpsimd.indirect_dma_start(
                out=vt[:, :P * HID], out_offset=None,
                in_=d_vb[:, :],
                in_offset=bass.IndirectOffsetOnAxis(
                    ap=idx_sb[:, gi:gi + 1], axis=0))
            v_sb.append(vt)

        # recip workspace
        rec2 = const.tile([1, 2 * TQ], F32, tag="rec2")
        rscr = const.tile([1, 2 * TQ], F32, tag="rscr")

        # attention outputs (divided), one tile per head-pair: [96, 2*TQ]
        apair = []
        for h2 in range(H // 2):
            apair.append(attnp.tile([DH, 2 * TQ], F32, tag=f"apair{h2}",
                                    name=f"apair{h2}"))

        # ================= attention main loop =================
        for hg in range(HG):
            # per-hg psum accumulators (live across st loop)
            numer = [psum1.tile([128, 2 * TQ], F32, space="PSUM",
                                tag=f"numer_{i}",
                                name=f"numer{hg}_{i}") for i in range(2)]
            den_ps = [psum1.tile([1, 2 * TQ], F32, space="PSUM",
                                 tag=f"den_{i}", name=f"den{hg}_{i}")
                      for i in range(2)]
            for st in range(NST):
                scores = psum.tile([128, 4 * TQ], F32, space="PSUM",
                                   tag="scores")
                # bias preload into PSUM (identity matmul, fp32r full rate)
                bt = biasp.tile([128, 4 * TQ], F16, tag="bias")
                from contextlib import nullcontext
                with (tc.high_priority() if (hg == 0 and st < 2)
                      else nullcontext()):
                    nc.sync.dma_start(
                        out=bt[:].rearrange("p (i t) -> p i t", i=4),
                        in_=d_bias[st * 128:(st + 1) * 128,
                                   hg * 4:hg * 4 + 4, :])
                for half in range(2):
                    nc.tensor.matmul(
                        out=scores[:, half * 512:(half + 1) * 512],
                        lhsT=ident[:],
                        rhs=bt[:, half * 512:(half + 1) * 512],
                        start=True, stop=False)
                # QK accumulate on top (i=1,3 close their banks)
                for i in range(4):
                    h = hg * 4 + i
                    nc.tensor.matmul(
                        out=scores[:, i * TQ:(i + 1) * TQ],
                        lhsT=kTe[:, h * S + st * 128:h * S + (st + 1) * 128],
                        rhs=qT[:, h * TQ:(h + 1) * TQ],
                        start=False, stop=(i % 2 == 1))
                # m = (scores + 20) * law    [one fat DVE pass]
                m = work.tile([128, 4 * TQ], F32, tag="m")
                law_st = law[:, st * TQ:(st + 1) * TQ]
                nc.vector.scalar_tensor_tensor(
                    out=m[:].rearrange("p (i t) -> p i t", i=4),
                    in0=scores[:].rearrange("p (i t) -> p i t", i=4),
                    scalar=SMOOTH, in1=law_st.unsqueeze(1).to_broadcast([128, 4, TQ]),
                    op0=ALU.add, op1=ALU.mult)
                # e = exp(m - 20) -> bf16
                e = work.tile([128, 4 * TQ], BF16, tag="e", bufs=3)
                nc.scalar.activation(e[:], m[:], AF.Exp, bias=neg20[:], scale=1.0)
                # g = e * law (bf16)
                g = work.tile([128, 4 * TQ], BF16, tag="g", bufs=3)
                lawb_st = lawb[:, st * TQ:(st + 1) * TQ]
                geng = nc.gpsimd if st % 2 == 0 else nc.vector
                geng.tensor_tensor(
                    out=g[:].rearrange("p (i t) -> p i t", i=4),
                    in0=e[:].rearrange("p (i t) -> p i t", i=4),
                    in1=lawb_st.unsqueeze(1).to_broadcast([128, 4, TQ]),
                    op=ALU.mult)
                # denominators: ones.T @ e -> [1, 512] per head-pair
                for j in range(2):
                    nc.tensor.matmul(
                        out=den_ps[j][0:1, :],
                        lhsT=ones_b[:],
                        rhs=e[:, j * 512:(j + 1) * 512],
                        start=(st == 0), stop=(st == NST - 1))
                # numerators: v_ext.T @ g -> [96, TQ] per head
                for i in range(4):
                    h = hg * 4 + i
                    nc.tensor.matmul(
                        out=numer[i // 2][:, (i % 2) * TQ:(i % 2 + 1) * TQ],
                        lhsT=v_sb[st][:, h * DH:h * DH + 128],
                        rhs=g[:, i * TQ:(i + 1) * TQ],
                        start=(st == 0 and i % 2 == 0),
                        stop=(st == NST - 1 and i % 2 == 1))
            # drain: free numer psum fast with plain copies; recip rows
            # bounce to DRAM off the critical path
            h0 = hg * 4
            for i in range(2):
                nc.scalar.copy(apair[hg * 2 + i][:], numer[i][:96, :])
                nc.vector.reciprocal_approx_fast(
                    out=rec2[:, :], in_=den_ps[i][0:1, :])
                nc.sync.dma_start(
                    out=d_rec[h0 + 2 * i:h0 + 2 * i + 2, :],
                    in_=rec2[:, :].rearrange("one (j t) -> one j t", j=2))

        # ============ remap heads -> channel-major tiles ============
        # attn_raw_ct[ci] : [128 (c%128), P*TQ], c = h*32+dd, free = (p, t)
        attn_rawct = []
        rcp_ct = []
        attn_ct = []
        for ci in range(4):
            raw = attnp.tile([128, P * TQ], F32, tag=f"raw{ci}", name=f"raw{ci}")
            attn_rawct.append(raw)
            rcp = attnp.tile([128, P * TQ], F32, tag=f"rcp{ci}", name=f"rcp{ci}")
            rcp_ct.append(rcp)
            act = attnp.tile([128, P * TQ], F16, tag=f"act{ci}", name=f"act{ci}")
            attn_ct.append(act)
        for h in range(H):
            ci, r0 = h // 4, (h % 4) * 32
            for p in range(P):
                eng = nc.sync if (h + p) % 2 == 0 else nc.scalar
                eng.dma_start(
                    out=attn_rawct[ci][r0:r0 + 32, p * TQ:(p + 1) * TQ],
                    in_=apair[h // 2][p * 32:p * 32 + 32,
                                      (h % 2) * TQ:(h % 2 + 1) * TQ])
            # recip broadcast rows for this head: [32, (p, t)]
            nc.gpsimd.dma_start(
                out=rcp_ct[ci][r0:r0 + 32, :].rearrange(
                    "dd (p t) -> dd p t", p=P),
                in_=d_rec[h:h + 1, :].unsqueeze(1).to_broadcast((32, P, TQ)))
        # divide (fp16 out for the out_proj lhsT)
        for ci in range(4):
            nc.vector.tensor_tensor(
                out=attn_ct[ci][:], in0=attn_rawct[ci][:],
                in1=rcp_ct[ci][:], op=ALU.mult)

        # ================= equivariant LN =================
        sqp = work.tile([128, P * TQ], F32R, tag="sq")
        ssq_a = psum1.tile([1, 512], F32, space="PSUM", tag="den_0")
        ssq_b = psum1.tile([1, TQ], F32, space="PSUM", tag="den_1")
        for ci in range(4):
            aci = attn_ct[ci][:]
            nc.vector.tensor_tensor(out=sqp[:], in0=aci, in1=aci, op=ALU.mult)
            nc.tensor.matmul(out=ssq_a[0:1, :], lhsT=ones_f[:].bitcast(F32R),
                             rhs=sqp[:, 0:512],
                             start=(ci == 0), stop=(ci == 3))
            nc.tensor.matmul(out=ssq_b[0:1, :], lhsT=ones_f[:].bitcast(F32R),
                             rhs=sqp[:, 512:768],
                             start=(ci == 0), stop=(ci == 3))
        # fold p-blocks: y[t] = ssq(p0)+ssq(p1)+ssq(p2)
        yrow = const.tile([1, TQ], F32, tag="yrow")
        nc.vector.tensor_copy(yrow[:], ssq_a[0:1, 0:TQ])
        nc.vector.tensor_tensor(out=yrow[:], in0=yrow[:],
                                in1=ssq_a[0:1, TQ:2 * TQ], op=ALU.add)
        nc.vector.tensor_tensor(out=yrow[:], in0=yrow[:], in1=ssq_b[0:1, :],
                                op=ALU.add)
        # inorm = rsqrt(y/512 + eps), then one Newton step
        # r' = r*(1.5 - 0.5*a*r^2) with a = y/512+eps
        arow = const.tile([1, TQ], F32, tag="arow")
        nc.vector.tensor_scalar(
            out=arow[:], in0=yrow[:], scalar1=1.0 / HID, scalar2=EPS,
            op0=ALU.mult, op1=ALU.add)
        rcpa = const.tile([1, TQ], F32, tag="rcpa")
        nc.vector.reciprocal_approx_fast(out=rcpa[:], in_=arow[:])
        r0t = const.tile([1, TQ], F32, tag="r0t")
        nc.scalar.activation(r0t[:], rcpa[:], AF.Sqrt, bias=0.0, scale=1.0)
        tmp = const.tile([1, TQ], F32, tag="tmpn")
        nc.vector.tensor_tensor(out=tmp[:], in0=r0t[:], in1=r0t[:], op=ALU.mult)
        nc.vector.tensor_tensor(out=tmp[:], in0=tmp[:], in1=arow[:], op=ALU.mult)
        nc.vector.tensor_scalar(
            out=tmp[:], in0=tmp[:], scalar1=-0.5, scalar2=1.5,
            op0=ALU.mult, op1=ALU.add)
        inorm = const.tile([1, TQ], F32R, tag="inorm")
        nc.vector.tensor_tensor(out=inorm[:], in0=r0t[:], in1=tmp[:],
                                op=ALU.mult)
        # inorm as columns [128,1] per t-half (bounce through DRAM)
        icol = const.tile([128, 2], F32, tag="icol")
        nc.sync.dma_start(out=d_ino.rearrange("(one t) -> one t", one=1),
                          in_=inorm[:, :].bitcast(F32))
        for th in range(2):
            nc.sync.dma_start(
                out=icol[:, th:th + 1],
                in_=d_ino[th * 128:(th + 1) * 128]
                    .rearrange("(p one) -> p one", one=1))

        # ================= out_proj =================
        nc.sync.dma_start(
            out=wT[:].rearrange("p (ci o) -> p ci o", ci=4),
            in_=d_wT.rearrange("(ci p) o -> p ci o", p=128))
        for k in range(6):          # tp-tiles: p = k//2, t-half = k%2
            op = psum.tile([128, HID], F32, space="PSUM", tag="scores")
            for ci in range(4):
                nc.tensor.matmul(
                    out=op[:, :],
                    lhsT=attn_ct[ci][:, k * 128:(k + 1) * 128],
                    rhs=wT[:, ci * HID:(ci + 1) * HID],
                    start=(ci == 0), stop=(ci == 3))
            ot = work.tile([128, HID], F32, tag="osb")
            nc.vector.tensor_scalar(
                out=ot[:], in0=op[:, :], scalar1=icol[:, k % 2:k % 2 + 1],
                scalar2=None, op0=ALU.mult)
            nc.sync.dma_start(
                out=d_out[(k % 2) * 128:(k % 2) * 128 + 128, k // 2, :],
                in_=ot[:])


def _host_prep(q, k, v, attn_bias, local_attention_weight, out_proj_w,
               ln_weight, outcell_index):
    """Pure layout marshalling on host -> per-core input dicts."""
    q = np.asarray(q, np.float32)
    k = np.asarray(k, np.float32)
    v = np.asarray(v, np.float32)
    attn_bias = np.asarray(attn_bias, np.float32)
    law = np.asarray(local_attention_weight, np.float32)
    out_proj_w = np.asarray(out_proj_w, np.float32)
    ln_weight = np.asarray(ln_weight, np.float32)
    idx = np.asarray(outcell_index).astype(np.int64)

    # (B,T,P,HID) -> (B, 96, H, T) with row j = p*32+dd
    def to_dT(x):
        return np.ascontiguousarray(
            x.reshape(B, T, P, H, D).transpose(0, 2, 4, 3, 1)
        ).reshape(B, P * D, H, T)

    qT = to_dT(q) * np.float32(SCALING)
    kT = to_dT(k)
    # K PBC expansion along token axis (gather columns)
    kTe = np.concatenate(
        [kT, np.take_along_axis(
            kT, idx[:, None, None, :].astype(np.int64), axis=3)], axis=3)
    biasT = np.ascontiguousarray(
        attn_bias.transpose(0, 3, 1, 2)).astype(np.float16)       # (B,S,H,T)
    lawT = np.ascontiguousarray(law.transpose(0, 2, 1))            # (B,S,T)
    lawTb = lawT.astype(ml_dtypes.bfloat16)
    # head-major V columns: (B, T, (h, p, dd)) so each head is contiguous
    vb = np.ascontiguousarray(
        v.reshape(B, T, P, H, D).transpose(0, 1, 3, 2, 4)
    ).reshape(B, T, P * HID).astype(ml_dtypes.bfloat16)
    wT = np.ascontiguousarray(out_proj_w.T) * ln_weight[:, None]   # (c,o)
    wT = np.ascontiguousarray(wT, np.float32).astype(np.float16)
    vidx = idx.astype(np.int32).reshape(B, 2, 128, 1)

    in_maps = []
    for c in range(NCORES):
        b, th = c // 2, c % 2
        t0 = th * TQ
        lawc = np.ascontiguousarray(lawT[b, :, t0:t0 + TQ])
        lawc = np.ascontiguousarray(
            lawc.reshape(NST, 128, TQ).transpose(1, 0, 2)).reshape(128, NST * TQ)
        lawcb = np.ascontiguousarray(lawTb[b, :, t0:t0 + TQ])
        lawcb = np.ascontiguousarray(
            lawcb.reshape(NST, 128, TQ).transpose(1, 0, 2)).reshape(128, NST * TQ)
        in_maps.append(dict(
            biasT=np.ascontiguousarray(biasT[b, :, :, t0:t0 + TQ]),
            lawT=lawc,
            lawTb=lawcb,
            qT=np.ascontiguousarray(
                qT[b, :, :, t0:t0 + TQ]).reshape(DH, H * TQ)
                .astype(ml_dtypes.bfloat16),
            kTe=np.ascontiguousarray(kTe[b]).reshape(DH, H * S)
                .astype(ml_dtypes.bfloat16),
            vb=np.ascontiguousarray(vb[b]),
            vidx=np.ascontiguousarray(vidx[b]),
            wT=wT,
            ident=np.eye(128, dtype=np.float16),
        ))
    return in_maps


def kernel(**inputs):
    global _CACHED_NC
    if _CACHED_NC is None:
        _CACHED_NC = build_nc()
    nc = _CACHED_NC
    in_maps = _host_prep(
        inputs["q"], inputs["k"], inputs["v"], inputs["attn_bias"],
        inputs["local_attention_weight"], inputs["out_proj_w"],
        inputs["ln_weight"], inputs["outcell_index"])
    res = run_bass_kernel_spmd(nc, in_maps, core_ids=list(range(NCORES)))
    out = np.empty((B, T, P, HID), np.float32)
    for c in range(NCORES):
        b, th = c // 2, c % 2
        out[b, th * TQ:(th + 1) * TQ] = res.results[c]["out"]
    return out


# revision 39
# speedup vs baseline: 1.1776x; 1.0034x over previous
"""Trainium2 Bass kernel for EquivariantAttention (sparse_attention).

Full (unsharded) inputs in, full output out. Internally shards over the 8
NeuronCores as (batch, T-half): core c handles batch b = c // 2, query rows
t0 = (c % 2) * 256 .. t0+256.  Every core runs the identical SPMD program on
its own input slices; there is no cross-core communication (LN and out_proj
are row-local in (b, t)).

Device-side per core:
  scores_T[s,t] = bias_T + (k_T.T @ q_T)          (bias preloaded into PSUM
                                                   via identity matmul, QK
                                                   accumulates on top, fp32r)
  m = (scores_T + 20) * law_T                     (one DVE pass, fp32)
  e = exp(m - 20)  -> bf16                        (ACT, free affine bias)
  g = e * law_T                                   (DVE, bf16)
  denom[t] = sum_s e                              (ones-column matmul)
  numer[j,t] = sum_s v_ext[s,j] * g[s,t]          (bf16 matmul; V PBC rows
                                                   gathered on device by
                                                   indirect DMA)
  attn = numer * recip(denom)                     (DVE)
  inorm = rsqrt(SSQ/512 + 1e-3)                   (SSQ via ones-matmul of
                                                   attn^2, ACT rsqrt +
                                                   1 Newton step)
  out = (attn @ (out_proj_w * ln_w).T) * inorm    (fp32r matmuls, inorm
                                                   folded into PSUM->SBUF
                                                   copy-out)
"""

import numpy as np
import ml_dtypes

import concourse.bass as bass
import concourse.bacc as bacc
import concourse.tile as tile
from concourse import mybir
from concourse.bass_utils import run_bass_kernel_spmd

# Problem constants (hardcoded per contract)
B, T, P, HID = 4, 512, 3, 512
H, D = 16, 32
EXP = 256
S = T + EXP            # 768
SCALING = (D / 3.0) ** 0.5 / D
SMOOTH = 20.0
EPS = 1e-3

NCORES = 8
TQ = T // 2            # 256 query rows per core
DH = P * D             # 96 head dim
NST = S // 128         # 6 s-tiles of 128
HG = 4                 # head groups of 4 heads

F32 = mybir.dt.float32
F16 = mybir.dt.float16
F32R = mybir.dt.float32r
BF16 = mybir.dt.bfloat16
I32 = mybir.dt.int32
AF = mybir.ActivationFunctionType
ALU = mybir.AluOpType

_CACHED_NC = None


def r(ap):
    """bitcast f32 AP -> f32r for full-rate PE."""
    return ap.bitcast(F32R)


def build_nc():
    nc = bacc.Bacc("TRN2", target_bir_lowering=False, debug=False)

    # ---- DRAM I/O (per-core shapes) ----
    d_bias = nc.dram_tensor("biasT", [S, H, TQ], F16, kind="ExternalInput").ap()
    d_law = nc.dram_tensor("lawT", [128, NST * TQ], F32, kind="ExternalInput").ap()
    d_lawb = nc.dram_tensor("lawTb", [128, NST * TQ], BF16, kind="ExternalInput").ap()
    d_qT = nc.dram_tensor("qT", [DH, H * TQ], BF16, kind="ExternalInput").ap()
    d_kTe = nc.dram_tensor("kTe", [DH, H * S], BF16, kind="ExternalInput").ap()
    d_vb = nc.dram_tensor("vb", [T, P * HID], BF16, kind="ExternalInput").ap()
    d_vidx = nc.dram_tensor("vidx", [2, 128, 1], I32, kind="ExternalInput").ap()
    d_wT = nc.dram_tensor("wT", [HID, HID], F16, kind="ExternalInput").ap()
    d_id = nc.dram_tensor("ident", [128, 128], F16, kind="ExternalInput").ap()
    d_out = nc.dram_tensor("out", [TQ, P, HID], F32, kind="ExternalOutput").ap()
    d_rec = nc.dram_tensor("rec_scratch", [16, TQ], F32).ap()
    d_ino = nc.dram_tensor("inorm_scratch", [TQ], F32).ap()

    with tile.TileContext(nc) as tc:
        build_kernel(tc, d_bias, d_law, d_lawb, d_qT, d_kTe, d_vb, d_vidx,
                     d_wT, d_id, d_out, d_rec, d_ino)
    nc.compile()
    return nc


def build_kernel(tc, d_bias, d_law, d_lawb, d_qT, d_kTe, d_vb, d_vidx,
                 d_wT, d_id, d_out, d_rec, d_ino):
    nc = tc.nc
    from contextlib import ExitStack
    ctx = ExitStack()
    with ctx:
        const = ctx.enter_context(tc.tile_pool(name="const", bufs=1))
        big = ctx.enter_context(tc.tile_pool(name="big", bufs=1))
        biasp = ctx.enter_context(tc.tile_pool(name="biasp", bufs=8))
        work = ctx.enter_context(tc.tile_pool(name="work", bufs=2))
        attnp = ctx.enter_context(tc.tile_pool(name="attnp", bufs=1))
        psum = ctx.enter_context(tc.tile_pool(name="psum", bufs=2, space="PSUM"))
        psum1 = ctx.enter_context(tc.tile_pool(name="psum1", bufs=1, space="PSUM"))

        # ---- constants ----
        ident = const.tile([128, 128], F16, tag="ident")
        nc.sync.dma_start(out=ident[:], in_=d_id)
        ones_b = const.tile([128, 1], BF16, tag="ones_b")
        nc.vector.memset(ones_b[:], 1.0)
        ones_f = const.tile([128, 1], F32, tag="ones_f")
        nc.vector.memset(ones_f[:], 1.0)
        neg20 = const.tile([128, 1], F32, tag="neg20")
        nc.vector.memset(neg20[:], -SMOOTH)
        ones_r = const.tile([1, 1], F32R, tag="ones_r")
        nc.vector.tensor_copy(ones_r[:], ones_f[0:1, 0:1])

        # ---- resident loads ----
        law = const.tile([128, NST * TQ], F32, tag="law")      # (s%128,(st,t))
        lawb = const.tile([128, NST * TQ], BF16, tag="lawb")
        qT = const.tile([DH, H * TQ], BF16, tag="qT")
        kTe = big.tile([DH, H * S], BF16, tag="kTe")
        # need-ordered: first head-group's k/q first, law early (stt dep)
        with tc.high_priority():
            nc.sync.dma_start(out=law[:], in_=d_law)
            nc.sync.dma_start(out=kTe[:, :4 * S], in_=d_kTe[:, :4 * S])
            nc.sync.dma_start(out=qT[:, :4 * TQ], in_=d_qT[:, :4 * TQ])
        nc.sync.dma_start(out=qT[:, 4 * TQ:8 * TQ],
                          in_=d_qT[:, 4 * TQ:8 * TQ])
        for c in range(1, 4):
            nc.sync.dma_start(
                out=kTe[:, c * 4 * S:(c + 1) * 4 * S],
                in_=d_kTe[:, c * 4 * S:(c + 1) * 4 * S])
            if c == 1:
                nc.sync.dma_start(out=lawb[:], in_=d_lawb)
                nc.sync.dma_start(out=qT[:, 8 * TQ:], in_=d_qT[:, 8 * TQ:])
        wT = const.tile([128, 4 * HID], F16, tag="wT")          # (c%128,(ci,o))

        # V tiles: 4 direct + 2 gathered (PBC expansion), bf16
        v_sb = []
        for st in range(4):
            vt = const.tile([128, P * HID + 128], BF16, tag=f"v{st}",
                            name=f"v{st}")
            nc.vector.memset(vt[:, P * HID:], 0.0)
            nc.sync.dma_start(out=vt[:, :P * HID],
                              in_=d_vb[st * 128:(st + 1) * 128, :])
            v_sb.append(vt)
        idx_sb = const.tile([128, 2], I32, tag="idx")
        nc.gpsimd.dma_start(
            out=idx_sb[:].rearrange("p (two one) -> p two one", one=1),
            in_=d_vidx.rearrange("two p one -> p two one"))
        for gi in range(2):
            vt = const.tile([128, P * HID + 128], BF16, tag=f"v{4 + gi}",
                            name=f"vg{gi}")
            nc.vector.memset(vt[:, P * HID:], 0.0)
            nc.gpsimd.indirect_dma_start(
                out=vt[:, :P * HID], out_offset=None,
                in_=d_vb[:, :],
                in_offset=bass.IndirectOffsetOnAxis(
                    ap=idx_sb[:, gi:gi + 1], axis=0))
            v_sb.append(vt)

        # recip workspace
        rec2 = const.tile([1, 2 * TQ], F32, tag="rec2")
        rscr = const.tile([1, 2 * TQ], F32, tag="rscr")

        # attention outputs (divided), one tile per head-pair: [96, 2*TQ]
        apair = []
        for h2 in range(H // 2):
            apair.append(attnp.tile([DH, 2 * TQ], F32, tag=f"apair{h2}",
                                    name=f"apair{h2}"))

        # ================= attention main loop =================
        for hg in range(HG):
            # per-hg psum accumulators (live across st loop)
            numer = [psum1.tile([128, 2 * TQ], F32, space="PSUM",
                                tag=f"numer_{i}",
                                name=f"numer{hg}_{i}") for i in range(2)]
            den_ps = [psum1.tile([1, 2 * TQ], F32, space="PSUM",
                                 tag=f"den_{i}", name=f"den{hg}_{i}")
                      for i in range(2)]
            for st in range(NST):
                scores = psum.tile([128, 4 * TQ], F32, space="PSUM",
                                   tag="scores")
                # bias preload into PSUM (identity matmul, fp32r full rate)
                bt = biasp.tile([128, 4 * TQ], F16, tag="bias")
                from contextlib import nullcontext
                with (tc.high_priority() if (hg == 0 and st < 2)
                      else nullcontext()):
                    nc.sync.dma_start(
                        out=bt[:].rearrange("p (i t) -> p i t", i=4),
                        in_=d_bias[st * 128:(st + 1) * 128,
                                   hg * 4:hg * 4 + 4, :])
                for half in range(2):
                    nc.tensor.matmul(
                        out=scores[:, half * 512:(half + 1) * 512],
                        lhsT=ident[:],
                        rhs=bt[:, half * 512:(half + 1) * 512],
                        start=True, stop=False)
                # QK accumulate on top (i=1,3 close their banks)
                for i in range(4):
                    h = hg * 4 + i
                    nc.tensor.matmul(
                        out=scores[:, i * TQ:(i + 1) * TQ],
                        lhsT=kTe[:, h * S + st * 128:h * S + (st + 1) * 128],
                        rhs=qT[:, h * TQ:(h + 1) * TQ],
                        start=False, stop=(i % 2 == 1))
                # m = (scores + 20) * law    [one fat DVE pass]
                m = work.tile([128, 4 * TQ], F32, tag="m", bufs=3)
                law_st = law[:, st * TQ:(st + 1) * TQ]
                nc.vector.scalar_tensor_tensor(
                    out=m[:].rearrange("p (i t) -> p i t", i=4),
                    in0=scores[:].rearrange("p (i t) -> p i t", i=4),
                    scalar=SMOOTH, in1=law_st.unsqueeze(1).to_broadcast([128, 4, TQ]),
                    op0=ALU.add, op1=ALU.mult)
                # e = exp(m - 20) -> bf16
                e = work.tile([128, 4 * TQ], BF16, tag="e", bufs=3)
                nc.scalar.activation(e[:], m[:], AF.Exp, bias=neg20[:], scale=1.0)
                # g = e * law (bf16)
                g = work.tile([128, 4 * TQ], BF16, tag="g", bufs=3)
                lawb_st = lawb[:, st * TQ:(st + 1) * TQ]
                geng = nc.gpsimd if st % 2 == 0 else nc.vector
                geng.tensor_tensor(
                    out=g[:].rearrange("p (i t) -> p i t", i=4),
                    in0=e[:].rearrange("p (i t) -> p i t", i=4),
                    in1=lawb_st.unsqueeze(1).to_broadcast([128, 4, TQ]),
                    op=ALU.mult)
                # denominators: ones.T @ e -> [1, 512] per head-pair
                for j in range(2):
                    nc.tensor.matmul(
                        out=den_ps[j][0:1, :],
                        lhsT=ones_b[:],
                        rhs=e[:, j * 512:(j + 1) * 512],
                        start=(st == 0), stop=(st == NST - 1))
                # numerators: v_ext.T @ g -> [96, TQ] per head
                for i in range(4):
                    h = hg * 4 + i
                    nc.tensor.matmul(
                        out=numer[i // 2][:, (i % 2) * TQ:(i % 2 + 1) * TQ],
                        lhsT=v_sb[st][:, h * DH:h * DH + 128],
                        rhs=g[:, i * TQ:(i + 1) * TQ],
                        start=(st == 0 and i % 2 == 0),
                        stop=(st == NST - 1 and i % 2 == 1))
            # drain: free numer psum fast with plain copies; recip rows
            # bounce to DRAM off the critical path
            h0 = hg * 4
            for i in range(2):
                nc.scalar.copy(apair[hg * 2 + i][:], numer[i][:96, :])
                nc.vector.reciprocal_approx_fast(
                    out=rec2[:, :], in_=den_ps[i][0:1, :])
                nc.sync.dma_start(
                    out=d_rec[h0 + 2 * i:h0 + 2 * i + 2, :],
                    in_=rec2[:, :].rearrange("one (j t) -> one j t", j=2))

        # ============ remap heads -> channel-major tiles ============
        # attn_raw_ct[ci] : [128 (c%128), P*TQ], c = h*32+dd, free = (p, t)
        attn_rawct = []
        rcp_ct = []
        attn_ct = []
        for ci in range(4):
            raw = attnp.tile([128, P * TQ], F32, tag=f"raw{ci}", name=f"raw{ci}")
            attn_rawct.append(raw)
            rcp = attnp.tile([128, P * TQ], F32, tag=f"rcp{ci}", name=f"rcp{ci}")
            rcp_ct.append(rcp)
            act = attnp.tile([128, P * TQ], F16, tag=f"act{ci}", name=f"act{ci}")
            attn_ct.append(act)
        for h in range(H):
            ci, r0 = h // 4, (h % 4) * 32
            for p in range(P):
                eng = nc.sync if (h + p) % 2 == 0 else nc.scalar
                eng.dma_start(
                    out=attn_rawct[ci][r0:r0 + 32, p * TQ:(p + 1) * TQ],
                    in_=apair[h // 2][p * 32:p * 32 + 32,
                                      (h % 2) * TQ:(h % 2 + 1) * TQ])
            # recip broadcast rows for this head: [32, (p, t)]
            nc.gpsimd.dma_start(
                out=rcp_ct[ci][r0:r0 + 32, :].rearrange(
                    "dd (p t) -> dd p t", p=P),
                in_=d_rec[h:h + 1, :].unsqueeze(1).to_broadcast((32, P, TQ)))
        # divide (fp16 out for the out_proj lhsT)
        for ci in range(4):
            nc.vector.tensor_tensor(
                out=attn_ct[ci][:], in0=attn_rawct[ci][:],
                in1=rcp_ct[ci][:], op=ALU.mult)

        # ================= equivariant LN =================
        sqp = work.tile([128, P * TQ], F32R, tag="sq")
        ssq_a = psum1.tile([1, 512], F32, space="PSUM", tag="den_0")
        ssq_b = psum1.tile([1, TQ], F32, space="PSUM", tag="den_1")
        for ci in range(4):
            aci = attn_ct[ci][:]
            nc.vector.tensor_tensor(out=sqp[:], in0=aci, in1=aci, op=ALU.mult)
            nc.tensor.matmul(out=ssq_a[0:1, :], lhsT=ones_f[:].bitcast(F32R),
                             rhs=sqp[:, 0:512],
                             start=(ci == 0), stop=(ci == 3))
            nc.tensor.matmul(out=ssq_b[0:1, :], lhsT=ones_f[:].bitcast(F32R),
                             rhs=sqp[:, 512:768],
                             start=(ci == 0), stop=(ci == 3))
        # fold p-blocks: y[t] = ssq(p0)+ssq(p1)+ssq(p2)
        yrow = const.tile([1, TQ], F32, tag="yrow")
        nc.vector.tensor_copy(yrow[:], ssq_a[0:1, 0:TQ])
        nc.vector.tensor_tensor(out=yrow[:], in0=yrow[:],
                                in1=ssq_a[0:1, TQ:2 * TQ], op=ALU.add)
        nc.vector.tensor_tensor(out=yrow[:], in0=yrow[:], in1=ssq_b[0:1, :],
                                op=ALU.add)
        # inorm = rsqrt(y/512 + eps), then one Newton step
        # r' = r*(1.5 - 0.5*a*r^2) with a = y/512+eps
        arow = const.tile([1, TQ], F32, tag="arow")
        nc.vector.tensor_scalar(
            out=arow[:], in0=yrow[:], scalar1=1.0 / HID, scalar2=EPS,
            op0=ALU.mult, op1=ALU.add)
        rcpa = const.tile([1, TQ], F32, tag="rcpa")
        nc.vector.reciprocal_approx_fast(out=rcpa[:], in_=arow[:])
        r0t = const.tile([1, TQ], F32, tag="r0t")
        nc.scalar.activation(r0t[:], rcpa[:], AF.Sqrt, bias=0.0, scale=1.0)
        tmp = const.tile([1, TQ], F32, tag="tmpn")
        nc.vector.tensor_tensor(out=tmp[:], in0=r0t[:], in1=r0t[:], op=ALU.mult)
        nc.vector.tensor_tensor(out=tmp[:], in0=tmp[:], in1=arow[:], op=ALU.mult)
        nc.vector.tensor_scalar(
            out=tmp[:], in0=tmp[:], scalar1=-0.5, scalar2=1.5,
            op0=ALU.mult, op1=ALU.add)
        inorm = const.tile([1, TQ], F32R, tag="inorm")
        nc.vector.tensor_tensor(out=inorm[:], in0=r0t[:], in1=tmp[:],
                                op=ALU.mult)
        # inorm as columns [128,1] per t-half (bounce through DRAM)
        icol = const.tile([128, 2], F32, tag="icol")
        nc.sync.dma_start(out=d_ino.rearrange("(one t) -> one t", one=1),
                          in_=inorm[:, :].bitcast(F32))
        for th in range(2):
            nc.sync.dma_start(
                out=icol[:, th:th + 1],
                in_=d_ino[th * 128:(th + 1) * 128]
                    .rearrange("(p one) -> p one", one=1))

        # ================= out_proj =================
        nc.sync.dma_start(
            out=wT[:].rearrange("p (ci o) -> p ci o", ci=4),
            in_=d_wT.rearrange("(ci p) o -> p ci o", p=128))
        for k in range(6):          # tp-tiles: p = k//2, t-half = k%2
            op = psum.tile([128, HID], F32, space="PSUM", tag="scores")
            for ci in range(4):
                nc.tensor.matmul(
                    out=op[:, :],
                    lhsT=attn_ct[ci][:, k * 128:(k + 1) * 128],
                    rhs=wT[:, ci * HID:(ci + 1) * HID],
                    start=(ci == 0), stop=(ci == 3))
            ot = work.tile([128, HID], F32, tag="osb")
            nc.vector.tensor_scalar(
                out=ot[:], in0=op[:, :], scalar1=icol[:, k % 2:k % 2 + 1],
                scalar2=None, op0=ALU.mult)
            nc.sync.dma_start(
                out=d_out[(k % 2) * 128:(k % 2) * 128 + 128, k // 2, :],
                in_=ot[:])


def _host_prep(q, k, v, attn_bias, local_attention_weight, out_proj_w,
               ln_weight, outcell_index):
    """Pure layout marshalling on host -> per-core input dicts."""
    q = np.asarray(q, np.float32)
    k = np.asarray(k, np.float32)
    v = np.asarray(v, np.float32)
    attn_bias = np.asarray(attn_bias, np.float32)
    law = np.asarray(local_attention_weight, np.float32)
    out_proj_w = np.asarray(out_proj_w, np.float32)
    ln_weight = np.asarray(ln_weight, np.float32)
    idx = np.asarray(outcell_index).astype(np.int64)

    # (B,T,P,HID) -> (B, 96, H, T) with row j = p*32+dd
    def to_dT(x):
        return np.ascontiguousarray(
            x.reshape(B, T, P, H, D).transpose(0, 2, 4, 3, 1)
        ).reshape(B, P * D, H, T)

    qT = to_dT(q) * np.float32(SCALING)
    kT = to_dT(k)
    # K PBC expansion along token axis (gather columns)
    kTe = np.concatenate(
        [kT, np.take_along_axis(
            kT, idx[:, None, None, :].astype(np.int64), axis=3)], axis=3)
    biasT = np.ascontiguousarray(
        attn_bias.transpose(0, 3, 1, 2)).astype(np.float16)       # (B,S,H,T)
    lawT = np.ascontiguousarray(law.transpose(0, 2, 1))            # (B,S,T)
    lawTb = lawT.astype(ml_dtypes.bfloat16)
    # head-major V columns: (B, T, (h, p, dd)) so each head is contiguous
    vb = np.ascontiguousarray(
        v.reshape(B, T, P, H, D).transpose(0, 1, 3, 2, 4)
    ).reshape(B, T, P * HID).astype(ml_dtypes.bfloat16)
    wT = np.ascontiguousarray(out_proj_w.T) * ln_weight[:, None]   # (c,o)
    wT = np.ascontiguousarray(wT, np.float32).astype(np.float16)
    vidx = idx.astype(np.int32).reshape(B, 2, 128, 1)

    in_maps = []
    for c in range(NCORES):
        b, th = c // 2, c % 2
        t0 = th * TQ
        lawc = np.ascontiguousarray(lawT[b, :, t0:t0 + TQ])
        lawc = np.ascontiguousarray(
            lawc.reshape(NST, 128, TQ).transpose(1, 0, 2)).reshape(128, NST * TQ)
        lawcb = np.ascontiguousarray(lawTb[b, :, t0:t0 + TQ])
        lawcb = np.ascontiguousarray(
            lawcb.reshape(NST, 128, TQ).transpose(1, 0, 2)).reshape(128, NST * TQ)
        in_maps.append(dict(
            biasT=np.ascontiguousarray(biasT[b, :, :, t0:t0 + TQ]),
            lawT=lawc,
            lawTb=lawcb,
            qT=np.ascontiguousarray(
                qT[b, :, :, t0:t0 + TQ]).reshape(DH, H * TQ)
                .astype(ml_dtypes.bfloat16),
            kTe=np.ascontiguousarray(kTe[b]).reshape(DH, H * S)
                .astype(ml_dtypes.bfloat16),
            vb=np.ascontiguousarray(vb[b]),
            vidx=np.ascontiguousarray(vidx[b]),
            wT=wT,
            ident=np.eye(128, dtype=np.float16),
        ))
    return in_maps


def kernel(**inputs):
    global _CACHED_NC
    if _CACHED_NC is None:
        _CACHED_NC = build_nc()
    nc = _CACHED_NC
    in_maps = _host_prep(
        inputs["q"], inputs["k"], inputs["v"], inputs["attn_bias"],
        inputs["local_attention_weight"], inputs["out_proj_w"],
        inputs["ln_weight"], inputs["outcell_index"])
    res = run_bass_kernel_spmd(nc, in_maps, core_ids=list(range(NCORES)))
    out = np.empty((B, T, P, HID), np.float32)
    for c in range(NCORES):
        b, th = c // 2, c % 2
        out[b, th * TQ:(th + 1) * TQ] = res.results[c]["out"]
    return out
